# revision 3
# baseline (speedup 1.0000x reference)
"""Differential multi-head attention kernel for Trainium2 (8 NeuronCores).

Strategy (per core, data-parallel over batch: 16 batches / 8 cores = 2 each):
  Phase 1: x -> xT (PE transpose); QT/Q2T/KT/K2T = W.T @ xT (fp32r matmuls,
           features on partitions); V -> augmented-V (bf16, ones column at
           col 0 so the dual-softmax denominators fall out of the ctx matmul).
  Phase 2: per head: scoresT[kp, q] (fp32r), exp on ACT -> bf16, ctx matmul
           (bf16) accumulating [1+64, q] = [denom; ctxT]; reciprocal +
           DMA partition-broadcast; combine ctx1*r1 - lam*ctx2*r2 on DVE
           with accum_out feeding GroupNorm stats.
  GroupNorm: stats via ones-matmul cross-partition reduce, rstd =
           exp(-0.5*ln(var+eps)) (keeps ACT on one table set), per-head
           affine apply.
  Phase 3: write c[b] = [768, 577] to DRAM scratch, read back as the
           reinterpreted [577, 768] view, PE-transpose, out = ctx @ Wo + bo.
"""
import numpy as np

import concourse.bass as bass
import concourse.tile as tile
from concourse import mybir, bacc
from concourse import bass_utils
from concourse.masks import make_identity

f32 = mybir.dt.float32
f32r = mybir.dt.float32r
bf16 = mybir.dt.bfloat16
AF = mybir.ActivationFunctionType
OP = mybir.AluOpType

B, S, D = 16, 577, 768
H, Dh = 12, 64
N_CORES = 8
BL = B // N_CORES          # local batches per core
SQ = 578                   # q padded (even, 290+288 chunks)
SK = 640                   # s/kp padded to 5*128
NK = D // 128              # 6 contraction chunks
NT = (S + 127) // 128      # 5 seq tiles
LAST = S - 4 * 128         # 65 rows in the last seq tile
EPS = 1e-5
GN_N = float(Dh * S)       # groupnorm slab size


def bcast_ap(row_ap, nrows):
    """Partition-broadcast AP: repeat a single-partition row over nrows (DMA src)."""
    return bass.AP(tensor=row_ap.tensor, offset=row_ap.offset,
                   ap=[list(row_ap.ap[0]), [0, nrows]] + [list(x) for x in row_ap.ap[1:]])


def build_program(lam: float, repeat: int = 1):
    nc = bacc.Bacc(trn_type="TRN2", target_bir_lowering=False, debug=False)

    x = nc.dram_tensor("x", [BL, S, D], f32, kind="ExternalInput").ap()
    Wq = nc.dram_tensor("Wq", [D, 2 * D], f32, kind="ExternalInput").ap()
    bq = nc.dram_tensor("bq", [2 * D], f32, kind="ExternalInput").ap()
    Wk = nc.dram_tensor("Wk", [D, 2 * D], f32, kind="ExternalInput").ap()
    bk = nc.dram_tensor("bk", [2 * D], f32, kind="ExternalInput").ap()
    Wv = nc.dram_tensor("Wv", [D, D], f32, kind="ExternalInput").ap()
    bv = nc.dram_tensor("bv", [D], f32, kind="ExternalInput").ap()
    Wo = nc.dram_tensor("Wo", [D, D], f32, kind="ExternalInput").ap()
    bo = nc.dram_tensor("bo", [D], f32, kind="ExternalInput").ap()
    gn_w = nc.dram_tensor("gn_w", [D], f32, kind="ExternalInput").ap()
    gn_b = nc.dram_tensor("gn_b", [D], f32, kind="ExternalInput").ap()
    out = nc.dram_tensor("out", [BL, S, D], f32, kind="ExternalOutput").ap()

    with tile.TileContext(nc) as tc:
        build_body(nc, tc, x, Wq, bq, Wk, bk, Wv, bv, Wo, bo, gn_w, gn_b, out, lam,
                   repeat=repeat)
    nc.compile()
    return nc


def build_body(nc, tc, x, Wq, bq, Wk, bk, Wv, bv, Wo, bo, gn_w, gn_b, out, lam,
               repeat=1):
    # ---------- singles ----------
    sing = tc.alloc_tile_pool(name="singles", bufs=1)
    ident = sing.tile([128, 128], f32, tag="ident", name="ident")
    make_identity(nc, ident)

    # packed fp32r constants (row 64: bv | bo | ones; rank-1 lhsT+rhs share the row)
    CW = 2 * D + 320
    consts = sing.tile([65, CW], f32r, tag="consts", name="consts")
    nc.gpsimd.dma_start(out=consts[64:65, 0:D], in_=bass.AP(tensor=bv.tensor, offset=bv.offset, ap=[[D, 1], [1, D]]))
    nc.gpsimd.dma_start(out=consts[64:65, D:2 * D], in_=bass.AP(tensor=bo.tensor, offset=bo.offset, ap=[[D, 1], [1, D]]))
    ones_f = sing.tile([65, 320], f32, tag="ones_f", name="ones_f")
    nc.gpsimd.memset(ones_f, 1.0)
    nc.scalar.copy(out=consts[64:65, 2 * D:CW], in_=ones_f[64:65, :])
    bv_r = consts[64:65, 0:D]
    bo_r = consts[64:65, D:2 * D]
    ones_v = consts[64:65, 2 * D:CW]  # pairs with bv/bo

    # ones64 for the stats cross-partition matmul
    ones64 = sing.tile([64, 1], f32, tag="ones64", name="ones64")
    nc.gpsimd.memset(ones64, 1.0)

    eps_t = sing.tile([1, 1], f32, tag="eps_t", name="eps_t")
    nc.gpsimd.memset(eps_t, EPS)

    # per-partition bias tiles for the Q/K copyback (bqT[p, c] = bq[c*128+p])
    bqT = sing.tile([128, 2 * NK], f32, tag="bqT", name="bqT")
    nc.sync.dma_start(out=bqT, in_=bass.AP(tensor=bq.tensor, offset=bq.offset,
                                           ap=[[1, 128], [128, 2 * NK]]))
    bkT = sing.tile([128, 2 * NK], f32, tag="bkT", name="bkT")
    nc.sync.dma_start(out=bkT, in_=bass.AP(tensor=bk.tensor, offset=bk.offset,
                                           ap=[[1, 128], [128, 2 * NK]]))

    # groupnorm params transposed: [64 d, 12 h] at partition rows 1:65
    gn_wT = sing.tile([64, H], f32, tag="gn_wT", name="gn_wT")
    nc.sync.dma_start(out=gn_wT,
                      in_=bass.AP(tensor=gn_w.tensor, offset=gn_w.offset,
                                  ap=[[1, 64], [64, H]]))
    gn_bT = sing.tile([64, H], f32, tag="gn_bT", name="gn_bT")
    nc.sync.dma_start(out=gn_bT,
                      in_=bass.AP(tensor=gn_b.tensor, offset=gn_b.offset,
                                  ap=[[1, 64], [64, H]]))

    # ---------- per-batch SBUF pools (slots reused across batches via tags) ----------
    big = tc.alloc_tile_pool(name="big", bufs=1)
    wpool = tc.alloc_tile_pool(name="wpool", bufs=2)
    wvpool = tc.alloc_tile_pool(name="wvpool", bufs=6)
    xnpool = tc.alloc_tile_pool(name="xnpool", bufs=2)
    epool = tc.alloc_tile_pool(name="epool", bufs=3)
    rpool = tc.alloc_tile_pool(name="rpool", bufs=2)
    cpool = tc.alloc_tile_pool(name="cpool", bufs=2)
    ctxpool = tc.alloc_tile_pool(name="ctxpool", bufs=12)
    stpool = tc.alloc_tile_pool(name="stpool", bufs=1)
    opool = tc.alloc_tile_pool(name="opool", bufs=2)
    drpool = tc.alloc_tile_pool(name="drpool", bufs=2, space="DRAM")

    for rep, b in [(r_, b_) for r_ in range(repeat) for b_ in range(BL)]:
        # ================= Phase 1a: load x, build xT =================
        xT = big.tile([128, NK, SK], f32r, tag="xT_ctxTT", name=f"xT_{rep}_{b}")
        xns = []
        with tc.tile_pool(name=f"ps_xp{rep}_{b}", bufs=4, space="PSUM") as xp:
            for t in range(NT):
                sz = 128 if t < NT - 1 else LAST
                xn = xnpool.tile([128, D], f32, tag="xn", name=f"xn_{rep}_{b}_{t}")
                if sz < 128:
                    nc.vector.memset(xn, 0.0)
                nc.sync.dma_start(out=xn[0:sz, :], in_=x[b, t * 128:t * 128 + sz, :])
                xns.append(xn)
                for k in range(NK):
                    tp = xp.tile([128, 128], f32, tag="xp", name=f"xp_{rep}_{b}_{t}_{k}")
                    nc.tensor.transpose(tp, xn[:, k * 128:(k + 1) * 128], ident)
                    nc.vector.tensor_copy(xT[:, k, t * 128:(t + 1) * 128], tp)

        # ================= Phase 1b: QT / Q2T / KT / K2T / V =================
        QT = big.tile([128, NK, SQ], f32r, tag="QT", name=f"QT_{rep}_{b}")
        Q2T = big.tile([128, NK, SQ], f32r, tag="Q2T", name=f"Q2T_{rep}_{b}")
        KT = big.tile([128, NK, SK], f32r, tag="KT", name=f"KT_{rep}_{b}")
        K2T = big.tile([128, NK, SK], f32r, tag="K2T", name=f"K2T_{rep}_{b}")
        vaug = big.tile([128, NT, H, 65], bf16, tag="vaug", name=f"vaug_{rep}_{b}")
        # ones column (col 64, denominator trick); padded seq rows of the last
        # tile are never read (ctx matmuls use K=65 there)
        nc.gpsimd.memset(vaug[:, 0:NT - 1, :, 64:65], 1.0)
        nc.vector.memset(vaug[0:LAST, NT - 1, :, 64:65], 1.0)

        with tc.tile_pool(name=f"ps_qkv{rep}_{b}", bufs=1, space="PSUM") as qp:
            # ---- QT / Q2T: q chunks (290, 288) over SQ ----
            for c in range(2 * NK):
                wt = wpool.tile([128, NK, 128], f32r, tag="wq", name=f"wq_{rep}_{b}_{c}")
                nc.gpsimd.dma_start(
                    out=wt,
                    in_=bass.AP(tensor=Wq.tensor, offset=Wq.offset + c * 128,
                                ap=[[2 * D, 128], [128 * 2 * D, NK], [1, 128]]))
                pA = qp.tile([128, 320], f32, tag="pA", bufs=2, name=f"qA_{rep}_{b}_{c}")
                pB = qp.tile([128, 320], f32, tag="pB", bufs=2, name=f"qB_{rep}_{b}_{c}")
                for k in range(NK):
                    nc.tensor.matmul(pA[:, 0:290], wt[:, k, :], xT[:, k, 0:290],
                                     start=(k == 0), stop=(k == NK - 1))
                    nc.tensor.matmul(pB[:, 0:288], wt[:, k, :], xT[:, k, 290:578],
                                     start=(k == 0), stop=(k == NK - 1))
                dst = QT if c < NK else Q2T
                cc = c % NK
                nc.vector.tensor_scalar(out=dst[:, cc, 0:290], in0=pA[:, 0:290],
                                        scalar1=bqT[:, c:c + 1], scalar2=None,
                                        op0=OP.add)
                nc.vector.tensor_scalar(out=dst[:, cc, 290:578], in0=pB[:, 0:288],
                                        scalar1=bqT[:, c:c + 1], scalar2=None,
                                        op0=OP.add)

            # ---- KT / K2T: s chunks (320, 320) over SK ----
            for c in range(2 * NK):
                wt = wpool.tile([128, NK, 128], f32r, tag="wq", name=f"wk_{rep}_{b}_{c}")
                nc.gpsimd.dma_start(
                    out=wt,
                    in_=bass.AP(tensor=Wk.tensor, offset=Wk.offset + c * 128,
                                ap=[[2 * D, 128], [128 * 2 * D, NK], [1, 128]]))
                pA = qp.tile([128, 320], f32, tag="pA", bufs=2, name=f"kA_{rep}_{b}_{c}")
                pB = qp.tile([128, 320], f32, tag="pB", bufs=2, name=f"kB_{rep}_{b}_{c}")
                for k in range(NK):
                    nc.tensor.matmul(pA, wt[:, k, :], xT[:, k, 0:320],
                                     start=(k == 0), stop=(k == NK - 1))
                    nc.tensor.matmul(pB, wt[:, k, :], xT[:, k, 320:640],
                                     start=(k == 0), stop=(k == NK - 1))
                dst = KT if c < NK else K2T
                cc = c % NK
                nc.vector.tensor_scalar(out=dst[:, cc, 0:320], in0=pA,
                                        scalar1=bkT[:, c:c + 1], scalar2=None,
                                        op0=OP.add)
                nc.vector.tensor_scalar(out=dst[:, cc, 320:640], in0=pB,
                                        scalar1=bkT[:, c:c + 1], scalar2=None,
                                        op0=OP.add)

            # ---- V (augmented, bf16): psum [s_tile, feat] ----
            wvs = []
            for k in range(NK):
                wv = wvpool.tile([128, D], f32r, tag="wv", name=f"wv_{rep}_{b}_{k}")
                nc.gpsimd.dma_start(out=wv, in_=Wv[k * 128:(k + 1) * 128, :])
                wvs.append(wv)
            for t in range(NT):
                sz = 128 if t < NT - 1 else LAST
                vA = qp.tile([128, 384], f32, tag="vA", bufs=2, name=f"vA_{rep}_{b}_{t}")
                vB = qp.tile([128, 384], f32, tag="vB", bufs=2, name=f"vB_{rep}_{b}_{t}")
                for k in range(NK):
                    nc.tensor.matmul(vA, xT[:, k, t * 128:(t + 1) * 128],
                                     wvs[k][:, 0:384], start=(k == 0), stop=False)
                    nc.tensor.matmul(vB, xT[:, k, t * 128:(t + 1) * 128],
                                     wvs[k][:, 384:768], start=(k == 0), stop=False)
                nc.tensor.matmul(vA, ones_v[0:1, 0:128], bv_r[0:1, 0:384],
                                 start=False, stop=True)
                nc.tensor.matmul(vB, ones_v[0:1, 0:128], bv_r[0:1, 384:768],
                                 start=False, stop=True)
                nc.vector.tensor_copy(vaug[0:sz, t, 0:6, 0:64],
                                      vA[0:sz].rearrange("p (h d) -> p h d", h=6))
                nc.vector.tensor_copy(vaug[0:sz, t, 6:12, 0:64],
                                      vB[0:sz].rearrange("p (h d) -> p h d", h=6))

        # ================= Phase 2: attention per head =================
        stats = stpool.tile([64, 2 * H], f32, tag="stats", name=f"stats_{rep}_{b}")
        nc.gpsimd.memset(stats, 0.0)
        ctx_full = ctxpool.tile([64, H, SQ], f32, tag="ctx_full", bufs=1,
                                name=f"ctx_full_{rep}_{b}")
        with tc.tile_pool(name=f"ps_att{rep}_{b}", bufs=1, space="PSUM") as ap_:
            for h in range(H):
                hb = (h % 2) * 64
                hc = h // 2
                c1 = ap_.tile([65, 1024], f32, tag="c1", name=f"c1_{rep}_{b}_{h}")
                c2 = ap_.tile([65, 1024], f32, tag="c2", name=f"c2_{rep}_{b}_{h}")
                for kp in range(NT):
                    ksl = slice(kp * 128, (kp + 1) * 128)
                    kpsz = 128 if kp < NT - 1 else LAST
                    e1 = epool.tile([128, SQ], bf16, tag="e1", name=f"e1_{rep}_{b}_{h}_{kp}")
                    e2 = epool.tile([128, SQ], bf16, tag="e2", name=f"e2_{rep}_{b}_{h}_{kp}")
                    for (sf, KTt, QTt, et) in ((0, KT, QT, e1), (1, K2T, Q2T, e2)):
                        sA = ap_.tile([128, 320], f32, tag="sA", bufs=2,
                                      name=f"sA_{rep}_{b}_{h}_{kp}_{sf}")
                        sB = ap_.tile([128, 320], f32, tag="sB", bufs=2,
                                      name=f"sB_{rep}_{b}_{h}_{kp}_{sf}")
                        nc.tensor.matmul(sA[:, 0:290], KTt[hb:hb + 64, hc, ksl],
                                         QTt[hb:hb + 64, hc, 0:290], start=True, stop=True)
                        nc.tensor.matmul(sB[:, 0:288], KTt[hb:hb + 64, hc, ksl],
                                         QTt[hb:hb + 64, hc, 290:578], start=True, stop=True)
                        nc.scalar.activation(out=et[0:kpsz, 0:290], in_=sA[0:kpsz, 0:290],
                                             func=AF.Exp, scale=0.125)
                        nc.scalar.activation(out=et[0:kpsz, 290:578], in_=sB[0:kpsz, 0:288],
                                             func=AF.Exp, scale=0.125)
                    nc.tensor.matmul(c1[:, 0:512], vaug[0:kpsz, kp, h, :], e1[0:kpsz, 0:512],
                                     start=(kp == 0), stop=False, skip_group_check=True)
                    nc.tensor.matmul(c1[:, 512:577], vaug[0:kpsz, kp, h, :], e1[0:kpsz, 512:577],
                                     start=(kp == 0), stop=(kp == NT - 1),
                                     skip_group_check=True)
                    nc.tensor.matmul(c2[:, 0:512], vaug[0:kpsz, kp, h, :], e2[0:kpsz, 0:512],
                                     start=(kp == 0), stop=False, skip_group_check=True)
                    nc.tensor.matmul(c2[:, 512:577], vaug[0:kpsz, kp, h, :], e2[0:kpsz, 512:577],
                                     start=(kp == 0), stop=(kp == NT - 1),
                                     skip_group_check=True)
                # drain ctx psum to SBUF early (frees banks for the next head,
                # and all-SBUF DVE ops run in 2x mode)
                c1s = cpool.tile([65, SQ], f32, tag="c1s", name=f"c1s_{rep}_{b}_{h}")
                c2s = cpool.tile([65, SQ], f32, tag="c2s", name=f"c2s_{rep}_{b}_{h}")
                nc.scalar.copy(out=c1s[:, 0:578], in_=c1[:, 0:578])
                nc.scalar.copy(out=c2s[:, 0:578], in_=c2[:, 0:578])
                r1 = rpool.tile([65, SQ], f32, tag="r1", name=f"r1_{rep}_{b}_{h}")
                r2 = rpool.tile([65, SQ], f32, tag="r2", name=f"r2_{rep}_{b}_{h}")
                nc.vector.reciprocal(out=r1[64:65, 0:577], in_=c1s[64:65, 0:577])
                nc.vector.reciprocal(out=r2[64:65, 0:577], in_=c2s[64:65, 0:577])
                r1b = rpool.tile([64, SQ], f32, tag="r1b", name=f"r1b_{rep}_{b}_{h}")
                r2b = rpool.tile([64, SQ], f32, tag="r2b", name=f"r2b_{rep}_{b}_{h}")
                nc.scalar.dma_start(out=r1b[:, 0:577], in_=bcast_ap(r1[64:65, 0:577], 64))
                nc.scalar.dma_start(out=r2b[:, 0:577], in_=bcast_ap(r2[64:65, 0:577], 64))
                # combine: ctxT = c1*r1 - lam*c2*r2
                ut = cpool.tile([65, SQ], f32, tag="ut", name=f"ut_{rep}_{b}_{h}")
                tt = cpool.tile([65, SQ], f32, tag="tt", name=f"tt_{rep}_{b}_{h}")
                ctxT = ctx_full[:, h, :]
                nc.vector.scalar_tensor_tensor(
                    out=ut[0:64, 0:577], in0=c2s[0:64, 0:577], scalar=-lam,
                    in1=r2b[0:64, 0:577], op0=OP.mult, op1=OP.mult)
                nc.vector.tensor_tensor(
                    out=tt[0:64, 0:577], in0=c1s[0:64, 0:577], in1=r1b[0:64, 0:577],
                    op=OP.mult)
                nc.vector.scalar_tensor_tensor(
                    out=ctxT[:, 0:577], in0=tt[0:64, 0:577], scalar=1.0,
                    in1=ut[0:64, 0:577], op0=OP.mult, op1=OP.add,
                    accum_out=stats[0:64, h:h + 1])
                sq = cpool.tile([65, SQ], f32, tag="ut", name=f"sq_{rep}_{b}_{h}")
                nc.vector.scalar_tensor_tensor(
                    out=sq[0:64, 0:577], in0=ctxT[:, 0:577], scalar=1.0,
                    in1=ctxT[:, 0:577], op0=OP.mult, op1=OP.mult,
                    accum_out=stats[0:64, H + h:H + h + 1])

        # ================= GroupNorm stats + apply =================
        scr = drpool.tile([D, S], f32, tag="scr", name=f"scr_{rep}_{b}")
        with tc.tile_pool(name=f"ps_gn{rep}_{b}", bufs=1, space="PSUM") as gp:
            sps = gp.tile([1, 2 * H], f32, tag="sps", name=f"sps_{rep}_{b}")
            nc.tensor.matmul(sps, ones64, stats, start=True, stop=True)
            ssb = stpool.tile([1, 2 * H], f32, tag="ssb", name=f"ssb_{rep}_{b}")
            nc.vector.tensor_copy(ssb, sps)
        mu = stpool.tile([1, H], f32, tag="mu", name=f"mu_{rep}_{b}")
        nc.vector.tensor_scalar(out=mu, in0=ssb[0:1, 0:H], scalar1=1.0 / GN_N,
                                scalar2=None, op0=OP.mult)
        musq = stpool.tile([1, H], f32, tag="musq", name=f"musq_{rep}_{b}")
        nc.vector.tensor_tensor(out=musq, in0=mu, in1=mu, op=OP.mult)
        var = stpool.tile([1, H], f32, tag="var", name=f"var_{rep}_{b}")
        nc.vector.scalar_tensor_tensor(out=var, in0=ssb[0:1, H:2 * H],
                                       scalar=1.0 / GN_N, in1=musq,
                                       op0=OP.mult, op1=OP.subtract)
        lnv = stpool.tile([1, H], f32, tag="lnv", name=f"lnv_{rep}_{b}")
        nc.scalar.activation(out=lnv, in_=var, func=AF.Ln, bias=eps_t, scale=1.0)
        rstd = stpool.tile([1, H], f32, tag="rstd", name=f"rstd_{rep}_{b}")
        nc.scalar.activation(out=rstd, in_=lnv, func=AF.Exp, scale=-0.5)
        mu_b = stpool.tile([64, H], f32, tag="mu_b", name=f"mu_b_{rep}_{b}")
        rstd_b = stpool.tile([64, H], f32, tag="rstd_b", name=f"rstd_b_{rep}_{b}")
        nc.scalar.dma_start(out=mu_b, in_=bcast_ap(mu[0:1, :], 64))
        nc.scalar.dma_start(out=rstd_b, in_=bcast_ap(rstd[0:1, :], 64))
        scale_all = stpool.tile([64, H], f32, tag="scale_all", name=f"scale_all_{rep}_{b}")
        nc.vector.tensor_tensor(out=scale_all, in0=rstd_b,
                                in1=gn_wT, op=OP.mult)
        bias_all = stpool.tile([64, H], f32, tag="bias_all", name=f"bias_all_{rep}_{b}")
        nc.vector.scalar_tensor_tensor(out=bias_all, in0=mu_b,
                                       scalar=-1.0, in1=scale_all,
                                       op0=OP.mult, op1=OP.mult)
        nc.vector.tensor_tensor(out=bias_all, in0=bias_all,
                                in1=gn_bT, op=OP.add)
        for h in range(H):
            nc.vector.tensor_scalar(out=ctx_full[:, h, 0:577], in0=ctx_full[:, h, 0:577],
                                    scalar1=scale_all[:, h:h + 1],
                                    scalar2=bias_all[:, h:h + 1],
                                    op0=OP.mult, op1=OP.add)
        nc.sync.dma_start(
            out=bass.AP(tensor=scr.tensor, offset=scr.offset,
                        ap=[[S, 64], [64 * S, H], [1, S]]),
            in_=ctx_full[:, :, 0:577])

        # ================= Phase 3: reinterpret + output projection =================
        ctxTT = big.tile([128, NK, SK], f32r, tag="xT_ctxTT", name=f"ctxTT_{rep}_{b}")
        with tc.tile_pool(name=f"ps_tp{rep}_{b}", bufs=4, space="PSUM") as tpp:
            for i in range(NT):
                sz = 128 if i < NT - 1 else LAST
                cn = xnpool.tile([128, D], f32, tag="cn", name=f"cn_{rep}_{b}_{i}")
                if sz < 128:
                    nc.vector.memset(cn, 0.0)
                nc.sync.dma_start(
                    out=cn[0:sz, :],
                    in_=bass.AP(tensor=scr.tensor, offset=scr.offset + i * 128 * D,
                                ap=[[D, sz], [1, D]]))
                for j in range(NK):
                    tp = tpp.tile([128, 128], f32, tag="tp", name=f"tp_{rep}_{b}_{i}_{j}")
                    nc.tensor.transpose(tp, cn[:, j * 128:(j + 1) * 128], ident)
                    nc.vector.tensor_copy(ctxTT[:, j, i * 128:(i + 1) * 128], tp)

        with tc.tile_pool(name=f"ps_o{rep}_{b}", bufs=1, space="PSUM") as op_:
            wos = []
            for k in range(NK):
                wo = wvpool.tile([128, D], f32r, tag="wv", name=f"wo_{rep}_{b}_{k}")
                nc.gpsimd.dma_start(out=wo, in_=Wo[k * 128:(k + 1) * 128, :])
                wos.append(wo)
            for i in range(NT):
                sz = 128 if i < NT - 1 else LAST
                oA = op_.tile([128, 384], f32, tag="oA", bufs=2, name=f"oA_{rep}_{b}_{i}")
                oB = op_.tile([128, 384], f32, tag="oB", bufs=2, name=f"oB_{rep}_{b}_{i}")
                for j in range(NK):
                    nc.tensor.matmul(oA, ctxTT[:, j, i * 128:(i + 1) * 128],
                                     wos[j][:, 0:384], start=(j == 0), stop=False)
                    nc.tensor.matmul(oB, ctxTT[:, j, i * 128:(i + 1) * 128],
                                     wos[j][:, 384:768], start=(j == 0), stop=False)
                nc.tensor.matmul(oA, ones_v[0:1, 0:128], bo_r[0:1, 0:384],
                                 start=False, stop=True)
                nc.tensor.matmul(oB, ones_v[0:1, 0:128], bo_r[0:1, 384:768],
                                 start=False, stop=True)
                ot = opool.tile([128, D], f32, tag="ot", name=f"ot_{rep}_{b}_{i}")
                nc.scalar.copy(out=ot[:, 0:384], in_=oA)
                nc.scalar.copy(out=ot[:, 384:768], in_=oB)
                nc.sync.dma_start(out=out[b, i * 128:i * 128 + sz, :], in_=ot[0:sz, :])

    for p in (drpool, opool, stpool, ctxpool, cpool, rpool, epool, xnpool, wvpool,
              wpool, big, sing):
        p.release()


_CACHE = {}
LAST_EXEC_NS = 0
LAST_TRACE = None


def _get_program(lam: float):
    key = round(float(lam), 8)
    if key not in _CACHE:
        _CACHE[key] = build_program(float(lam))
    return _CACHE[key]


def kernel(**inputs):
    x = np.ascontiguousarray(np.asarray(inputs["x"], dtype=np.float32))
    lam = float(np.asarray(inputs["lam"]))
    nc = _get_program(lam)
    names = ["Wq", "bq", "Wk", "bk", "Wv", "bv", "Wo", "bo", "gn_w", "gn_b"]
    shared = {n: np.ascontiguousarray(np.asarray(inputs[n], dtype=np.float32))
              for n in names}
    in_maps = []
    for c in range(N_CORES):
        m = dict(shared)
        m["x"] = x[c * BL:(c + 1) * BL]
        in_maps.append(m)
    res = bass_utils.run_bass_kernel_spmd(nc, in_maps, list(range(N_CORES)))
    global LAST_EXEC_NS, LAST_TRACE
    if getattr(res, "exec_time_ns", None):
        LAST_EXEC_NS = res.exec_time_ns
        LAST_TRACE = getattr(res, "instructions_and_trace", None)
    return np.concatenate([res.results[c]["out"] for c in range(N_CORES)], axis=0)



# revision 38
# speedup vs baseline: 1.0495x; 1.0495x over previous
"""Differential multi-head attention kernel for Trainium2 (8 NeuronCores).

Data-parallel over batch (16/8 = 2 per core). Per core, software-pipelined:

  init:  weights cast to bf16 once (Wq/Wk to DRAM scratch in a head-paired
         layout: head h's stationary cols are [q1|q2], so the dual-softmax
         score matmuls row-pack into PE array halves 0:64 / 64:128 and run
         concurrently). Wq loads go first so batch-0 Q-proj starts early.
  P1(b): x -> bf16 -> PE transpose -> xT; Q/K projections write Q12/K12
         (head h: side1 on partitions 0:64, side2 on 64:128); V -> vaug
         (ones col 64 makes the softmax denominators fall out of ctx MMs).
  P2(b): per head: row-packed score MMs into a merged [128,2,1024] psum,
         ONE exp per (h,kp) for both sides (ACT, bf16 out), ctx MMs
         accumulate [65, S]; ctx drained on DVE (row 64 = denominators,
         gathered into 32-aligned quadrants of den_all).
  tail(b), per 6-head group: reciprocals, -lam fold, bf16 broadcast,
         combines on DVE (stats via accum_out), per-group GroupNorm
         (rstd = exp(-0.5 ln(var+eps)) keeps ACT on one table set),
         apply, per-group scratch write.
  P3(b): read the bf16 scratch reinterpreted [S, D], PE transpose ->
         ctxTT, out = ctxTT.T @ Wo + bo. t-tiles 0:2 only need head
         group 0, so they start before group 1 finishes.

  Emission interleave: P1(b+1) fills the PE during P2(b); tail(0)/P3(0)
  and tail(1)-group0 fill DVE/PE during P2(1).
"""
import numpy as np

import concourse.bass as bass
import concourse.tile as tile
from concourse import mybir, bacc
from concourse import bass_utils
from concourse.masks import make_identity

f32 = mybir.dt.float32
bf16 = mybir.dt.bfloat16
AF = mybir.ActivationFunctionType
OP = mybir.AluOpType

B, S, D = 16, 577, 768
H, Dh = 12, 64
N_CORES = 8
BL = B // N_CORES
NK = D // 128              # 6 contraction chunks
NT = (S + 127) // 128      # 5 seq tiles
LAST = S - 4 * 128         # 65
SQ = 578
EPS = 1e-5
GN_N = float(Dh * S)
KW = [128, 128, 128, 128, LAST]


def bcast_ap(row_ap, nrows):
    """Partition-broadcast AP: repeat a single-partition row over nrows."""
    return bass.AP(tensor=row_ap.tensor, offset=row_ap.offset,
                   ap=[list(row_ap.ap[0]), [0, nrows]] + [list(x) for x in row_ap.ap[1:]])


def build_program(lam: float):
    nc = bacc.Bacc(trn_type="TRN2", target_bir_lowering=False, debug=False)

    x = nc.dram_tensor("x", [BL, S, D], f32, kind="ExternalInput").ap()
    Wq = nc.dram_tensor("Wq", [D, 2 * D], f32, kind="ExternalInput").ap()
    bq = nc.dram_tensor("bq", [2 * D], f32, kind="ExternalInput").ap()
    Wk = nc.dram_tensor("Wk", [D, 2 * D], f32, kind="ExternalInput").ap()
    bk = nc.dram_tensor("bk", [2 * D], f32, kind="ExternalInput").ap()
    Wv = nc.dram_tensor("Wv", [D, D], f32, kind="ExternalInput").ap()
    bv = nc.dram_tensor("bv", [D], f32, kind="ExternalInput").ap()
    Wo = nc.dram_tensor("Wo", [D, D], f32, kind="ExternalInput").ap()
    bo = nc.dram_tensor("bo", [D], f32, kind="ExternalInput").ap()
    gn_w = nc.dram_tensor("gn_w", [D], f32, kind="ExternalInput").ap()
    gn_b = nc.dram_tensor("gn_b", [D], f32, kind="ExternalInput").ap()
    out = nc.dram_tensor("out", [BL, S, D], f32, kind="ExternalOutput").ap()

    with tile.TileContext(nc) as tc:
        build_body(nc, tc, x, Wq, bq, Wk, bk, Wv, bv, Wo, bo, gn_w, gn_b, out, lam)
    nc.compile()
    return nc


def build_body(nc, tc, x, Wq, bq, Wk, bk, Wv, bv, Wo, bo, gn_w, gn_b, out, lam):
    sing = tc.alloc_tile_pool(name="sing", bufs=1)
    big = tc.alloc_tile_pool(name="big", bufs=1)
    wqk = tc.alloc_tile_pool(name="wqk", bufs=4)
    xpool = tc.alloc_tile_pool(name="xpool", bufs=2)
    epool = tc.alloc_tile_pool(name="epool", bufs=3)
    cpool = tc.alloc_tile_pool(name="cpool", bufs=13)
    tpool = tc.alloc_tile_pool(name="tpool", bufs=2)
    rpool = tc.alloc_tile_pool(name="rpool", bufs=2)
    spool = tc.alloc_tile_pool(name="spool", bufs=1)
    drpool = tc.alloc_tile_pool(name="drpool", bufs=1, space="DRAM")
    ps = tc.alloc_tile_pool(name="ps", bufs=1, space="PSUM")

    # single "sc" slot (4 banks): merged scores [128, 2, 1024] f32; other
    # users take [128, 768] f32 or [128, 1536] bf16 views of the same slot.
    def sc_tile(name, shape=(128, 768), dtype=f32):
        return ps.tile(list(shape), dtype, tag="sc", bufs=1, name=name,
                       padded_shape=None)

    def s12_tile(name):
        return ps.tile([128, 2, 1024], f32, tag="sc", bufs=1, name=name)

    def ctx_tile(name):
        return ps.tile([65, 640], f32, tag="ctx", bufs=2, name=name)

    # ---------------- singles ----------------
    ones64 = sing.tile([64, 1], f32, tag="ones64", name="ones64")
    nc.gpsimd.memset(ones64, 1.0)
    onesrow = sing.tile([1, 128], bf16, tag="onesrow", name="onesrow")
    nc.gpsimd.memset(onesrow, 1.0)
    eps_t = sing.tile([1, 1], f32, tag="eps_t", name="eps_t")
    nc.gpsimd.memset(eps_t, EPS)
    ident = sing.tile([128, 128], bf16, tag="ident", name="ident")
    make_identity(nc, ident)

    # head-paired biases: bqT12[p, h] = bq[64h+p] (p<64) | bq[D+64h+p-64]
    bqT12 = sing.tile([128, H], f32, tag="bqT12", name="bqT12")
    bkT12 = sing.tile([128, H], f32, tag="bkT12", name="bkT12")
    for bt, src in ((bqT12, bq), (bkT12, bk)):
        nc.sync.dma_start(out=bt[0:64, :],
                          in_=bass.AP(tensor=src.tensor, offset=src.offset,
                                      ap=[[1, 64], [64, H]]))
        nc.sync.dma_start(out=bt[64:128, :],
                          in_=bass.AP(tensor=src.tensor, offset=src.offset + D,
                                      ap=[[1, 64], [64, H]]))
    gn_wT = sing.tile([64, H], f32, tag="gn_wT", name="gn_wT")
    nc.sync.dma_start(out=gn_wT, in_=bass.AP(tensor=gn_w.tensor, offset=gn_w.offset,
                                             ap=[[1, 64], [64, H]]))
    gn_bT = sing.tile([64, H], f32, tag="gn_bT", name="gn_bT")
    nc.sync.dma_start(out=gn_bT, in_=bass.AP(tensor=gn_b.tensor, offset=gn_b.offset,
                                             ap=[[1, 64], [64, H]]))

    # bias rows -> bf16
    bvo16 = sing.tile([1, 2 * D], bf16, tag="bvo16", name="bvo16")
    for i, src in enumerate((bv, bo)):
        bt = xpool.tile([1, D], f32, tag="xn", name=f"bt{i}")
        nc.gpsimd.dma_start(out=bt,
                            in_=bass.AP(tensor=src.tensor, offset=src.offset,
                                        ap=[[D, 1], [1, D]]))
        nc.vector.tensor_copy(bvo16[0:1, i * D:(i + 1) * D], bt)
    bvb = bvo16[0:1, 0:D]
    bob = bvo16[0:1, D:2 * D]

    # Wv / Wo resident bf16; Wq / Wk -> bf16 DRAM scratch, head-paired
    # [k, p, h, side, 64]. Wq first (unblocks batch-0 Q-proj), Wo last.
    WvB = sing.tile([128, NK, D], bf16, tag="WvB", name="WvB")
    WoB = sing.tile([128, NK, D], bf16, tag="WoB", name="WoB")
    WqB = drpool.tile([NK, 128, H, 2, 64], bf16, tag="WqB", name="WqB")
    WkB = drpool.tile([NK, 128, H, 2, 64], bf16, tag="WkB", name="WkB")

    def emit_w_prep():
        qs = [nc.scalar, nc.sync]
        qi = 0

        def qk_prep(dstW, srcW, tagn):
            nonlocal qi
            for k in range(NK):
                for s in range(2):
                    wt = xpool.tile([128, D], f32, tag="ot", name=f"w{tagn}_{k}_{s}")
                    qs[qi % 2].dma_start(
                        out=wt, in_=srcW[k * 128:(k + 1) * 128, s * D:(s + 1) * D])
                    wc = xpool.tile([128, D], bf16, tag="xb", name=f"w{tagn}c_{k}_{s}")
                    nc.vector.tensor_copy(wc, wt)
                    nc.gpsimd.dma_start(
                        out=dstW[k][:, :, s, :],
                        in_=wc.rearrange("p (h c) -> p h c", h=H))
                    qi += 1

        def vo_prep(dstW, srcW, tagn):
            nonlocal qi
            for k in range(NK):
                wt = xpool.tile([128, D], f32, tag="ot", name=f"w{tagn}_{k}")
                qs[qi % 2].dma_start(out=wt, in_=srcW[k * 128:(k + 1) * 128, :])
                nc.vector.tensor_copy(dstW[:, k, :], wt)
                qi += 1

        qk_prep(WqB, Wq, "q")
        vo_prep(WvB, Wv, "v")
        qk_prep(WkB, Wk, "k")
        vo_prep(WoB, Wo, "o")

    # per-batch persistent tiles
    xT = [big.tile([128, NK, 640], bf16, tag=f"xT{b}", name=f"xT{b}") for b in range(BL)]
    Q12 = [big.tile([128, H, SQ], bf16, tag=f"Q12_{b}", name=f"Q12_{b}") for b in range(BL)]
    K12 = [big.tile([128, H, SQ], bf16, tag=f"K12_{b}", name=f"K12_{b}") for b in range(BL)]
    vaug = [big.tile([128, NT, H, 65], bf16, tag=f"vaug{b}", name=f"vaug{b}") for b in range(BL)]
    ctxf = [big.tile([65, H, SQ], bf16, tag=f"ctxf{b}", name=f"ctxf{b}") for b in range(BL)]
    # den_all quadrants (32-aligned for DVE partition-base rules):
    # head group g = h // 6, side s: row = 64*g + 32*s + (h % 6)
    den_all = [spool.tile([102, SQ], bf16, tag=f"den{b}", name=f"den{b}") for b in range(BL)]
    stats = [spool.tile([64, 2 * H], f32, tag=f"stats{b}", name=f"stats{b}") for b in range(BL)]
    csh2 = [[None] * H for _ in range(BL)]
    r16 = [spool.tile([102, SQ], bf16, tag=f"r16_{b}", name=f"r16_{b}") for b in range(BL)]
    scr = [drpool.tile([608, D], bf16, tag=f"scr{b}", name=f"scr{b}") for b in range(BL)]

    for b in range(BL):
        nc.gpsimd.memset(vaug[b][:, 0:NT - 1, :, 64:65], 1.0)
        nc.gpsimd.memset(vaug[b][0:LAST, NT - 1, :, 64:65], 1.0)

    # zero-fill scratch pad rows (577:608) so P3 transposes read finite data
    zpad = xpool.tile([128, D], bf16, tag="xb", name="zpad")
    nc.vector.memset(zpad, 0.0)
    for b in range(BL):
        nc.gpsimd.dma_start(out=scr[b][S:608, :], in_=zpad[0:608 - S, :])

    # ---------------- phase emitters ----------------
    def p1_thunks(b):
        th = []

        def x_thunk(t):
            def f():
                sz = 128 if t < NT - 1 else LAST
                xn = xpool.tile([128, D], f32, tag="xn", name=f"xn{b}_{t}")
                nc.gpsimd.dma_start(out=xn[0:sz, :], in_=x[b, t * 128:t * 128 + sz, :])
                xb = xpool.tile([128, D], bf16, tag="xb", name=f"xb{b}_{t}")
                if sz < 128:
                    nc.vector.memset(xb, 0.0)
                nc.vector.tensor_copy(xb[0:sz, :], xn[0:sz, :])
                tp = sc_tile(f"tpx{b}_{t}", (128, 1536), bf16)
                for k in range(NK):
                    nc.tensor.transpose(tp[:, k * 128:(k + 1) * 128],
                                        xb[:, k * 128:(k + 1) * 128], ident)
                nc.vector.tensor_copy(
                    xT[b][:, 0:NK, t * 128:(t + 1) * 128],
                    tp[:, 0:768].rearrange("p (k c) -> p k c", k=NK))
            return f

        def qk_thunk(h, WB, dstT, biasT, nm):
            def f():
                wq = wqk.tile([128, NK, 2, 64], bf16, tag="wqk", name=f"w{nm}{b}_{h}")
                nc.gpsimd.dma_start(
                    out=wq,
                    in_=bass.AP(tensor=WB.tensor, offset=WB.offset + h * 128,
                                ap=[[H * 128, 128], [128 * H * 128, NK], [1, 128]]))
                q_ps = sc_tile(f"ps{nm}{b}_{h}")
                for k in range(NK):
                    nc.tensor.matmul(q_ps[:, 0:512], wq[:, k], xT[b][:, k, 0:512],
                                     start=(k == 0), stop=(k == NK - 1),
                                     skip_group_check=True)
                    nc.tensor.matmul(q_ps[:, 512:577], wq[:, k], xT[b][:, k, 512:577],
                                     start=(k == 0), stop=(k == NK - 1),
                                     skip_group_check=True)
                nc.vector.tensor_scalar(out=dstT[b][:, h, 0:577], in0=q_ps[:, 0:577],
                                        scalar1=biasT[:, h:h + 1], scalar2=None,
                                        op0=OP.add)
            return f

        def v_thunk(t):
            def f():
                sz = 128 if t < NT - 1 else LAST
                v_ps = sc_tile(f"psv{b}_{t}")
                for k in range(NK):
                    nc.tensor.matmul(v_ps[:, 0:512], xT[b][:, k, t * 128:(t + 1) * 128],
                                     WvB[:, k, 0:512], start=(k == 0), stop=False,
                                     skip_group_check=True)
                    nc.tensor.matmul(v_ps[:, 512:768], xT[b][:, k, t * 128:(t + 1) * 128],
                                     WvB[:, k, 512:768], start=(k == 0), stop=False,
                                     skip_group_check=True)
                nc.tensor.matmul(v_ps[:, 0:512], onesrow, bvb[0:1, 0:512],
                                 start=False, stop=True, skip_group_check=True)
                nc.tensor.matmul(v_ps[:, 512:768], onesrow, bvb[0:1, 512:768],
                                 start=False, stop=True, skip_group_check=True)
                nc.vector.tensor_copy(vaug[b][0:sz, t, 0:6, 0:64],
                                      v_ps[0:sz, 0:384].rearrange("p (h d) -> p h d", h=6))
                nc.vector.tensor_copy(vaug[b][0:sz, t, 6:12, 0:64],
                                      v_ps[0:sz, 384:768].rearrange("p (h d) -> p h d", h=6))
            return f

        for t in range(NT):
            th.append(x_thunk(t))
        for h in range(H):
            th.append(qk_thunk(h, WqB, Q12, bqT12, "q"))
        for h in range(H):
            th.append(qk_thunk(h, WkB, K12, bkT12, "k"))
        for t in range(NT):
            th.append(v_thunk(t))
        return th

    def attn_thunks(b):
        th = []

        def head_thunk(h):
            def f():
                c1 = ctx_tile(f"c1_{b}_{h}")
                c2 = ctx_tile(f"c2_{b}_{h}")
                for kp in range(NT):
                    kw = KW[kp]
                    ksl = slice(kp * 128, kp * 128 + kw)
                    e = epool.tile([128, 2, SQ], bf16, tag="e", name=f"e{b}_{h}_{kp}")
                    s12 = s12_tile(f"s12_{b}_{h}_{kp}")
                    for side in range(2):
                        off = side * 64
                        nc.tensor.matmul(s12[0:kw, side, 0:512],
                                         K12[b][off:off + 64, h, ksl],
                                         Q12[b][off:off + 64, h, 0:512],
                                         start=True, stop=True, skip_group_check=True)
                        nc.tensor.matmul(s12[0:kw, side, 512:577],
                                         K12[b][off:off + 64, h, ksl],
                                         Q12[b][off:off + 64, h, 512:577],
                                         start=True, stop=True, skip_group_check=True)
                    nc.scalar.activation(out=e[0:kw, :, 0:577],
                                         in_=s12[0:kw, :, 0:577],
                                         func=AF.Exp, scale=0.125)
                    for side, c in ((0, c1), (1, c2)):
                        nc.tensor.matmul(c[:, 0:512], vaug[b][0:kw, kp, h, :],
                                         e[0:kw, side, 0:512],
                                         start=(kp == 0), stop=False,
                                         skip_group_check=True)
                        nc.tensor.matmul(c[:, 512:577], vaug[b][0:kw, kp, h, :],
                                         e[0:kw, side, 512:577],
                                         start=(kp == 0), stop=(kp == NT - 1),
                                         skip_group_check=True)
                nc.vector.tensor_copy(ctxf[b][0:65, h, 0:577], c1[0:65, 0:577])
                ch2 = cpool.tile([65, SQ], bf16, tag="csh2", name=f"csh2_{b}_{h}")
                csh2[b][h] = ch2
                nc.vector.tensor_copy(ch2[0:65, 0:577], c2[0:65, 0:577])
                g = h // 6
                if h % 3 == 2:
                    # batched side-1 denominator gather for heads h-2..h
                    r0 = 64 * g + (h % 6) - 2
                    nc.sync.dma_start(out=den_all[b][r0:r0 + 3, 0:577],
                                      in_=ctxf[b][64:65, h - 2:h + 1, 0:577])
                nc.gpsimd.dma_start(
                    out=den_all[b][64 * g + 32 + (h % 6):64 * g + 33 + (h % 6), 0:577],
                    in_=ch2[64:65, 0:577])
            return f

        for h in range(H):
            th.append(head_thunk(h))
        return th

    def tail_thunks(b, g):
        """Tail for head group g (heads 6g..6g+5): recip, combines, GN, apply,
        scratch write. Returns a thunk list."""
        th = []
        q0 = 64 * g

        def recip():
            r_all = spool.tile([102, SQ], f32, tag=f"rall{b}", bufs=2,
                               name=f"rall{b}_{g}")
            nc.vector.reciprocal(out=r_all[q0:q0 + 6, 0:577],
                                 in_=den_all[b][q0:q0 + 6, 0:577])
            nc.vector.reciprocal(out=r_all[q0 + 32:q0 + 38, 0:577],
                                 in_=den_all[b][q0 + 32:q0 + 38, 0:577])
            nc.vector.tensor_copy(r16[b][q0:q0 + 6, 0:577], r_all[q0:q0 + 6, 0:577])
            nc.vector.tensor_scalar(out=r16[b][q0 + 32:q0 + 38, 0:577],
                                    in0=r_all[q0 + 32:q0 + 38, 0:577],
                                    scalar1=-lam, scalar2=None, op0=OP.mult)
        th.append(recip)

        def combine(h):
            def f():
                rb = rpool.tile([64, 2, SQ], bf16, tag="rb", name=f"rb{b}_{h}")
                for side in range(2):
                    row = q0 + 32 * side + (h % 6)
                    nc.sync.dma_start(out=rb[:, side, 0:577],
                                      in_=bcast_ap(r16[b][row:row + 1, 0:577], 64))
                tmp = tpool.tile([64, SQ], bf16, tag="tmp", name=f"tmp{b}_{h}")
                ch = ctxf[b][0:64, h, 0:577]
                ch2 = csh2[b][h]
                nc.vector.tensor_tensor(out=tmp[:, 0:577], in0=ch, in1=rb[:, 0, 0:577],
                                        op=OP.mult)
                nc.vector.tensor_tensor(out=ch2[0:64, 0:577], in0=ch2[0:64, 0:577],
                                        in1=rb[:, 1, 0:577], op=OP.mult)
                nc.vector.scalar_tensor_tensor(out=ch, in0=tmp[:, 0:577], scalar=1.0,
                                               in1=ch2[0:64, 0:577],
                                               op0=OP.mult, op1=OP.add,
                                               accum_out=stats[b][:, h:h + 1])
                nc.vector.scalar_tensor_tensor(out=tmp[:, 0:577], in0=ch, scalar=1.0,
                                               in1=ch, op0=OP.mult, op1=OP.mult,
                                               accum_out=stats[b][:, H + h:H + h + 1])
            return f
        for h in range(6 * g, 6 * g + 6):
            th.append(combine(h))

        def gn_apply():
            sps = ctx_tile(f"gn{b}_{g}")
            stats_g = stats[b].rearrange("p (a c) -> p a c", a=2)[:, :, 6 * g:6 * g + 6]
            nc.tensor.matmul(sps[0:1, 0:12], ones64, stats_g, start=True, stop=True,
                             skip_group_check=True)
            ssb = spool.tile([1, 12], f32, tag=f"ssb{b}", bufs=2, name=f"ssb{b}_{g}")
            nc.vector.tensor_copy(ssb, sps[0:1, 0:12])
            mu = spool.tile([1, 6], f32, tag=f"mu{b}", bufs=2, name=f"mu{b}_{g}")
            nc.vector.tensor_scalar(out=mu, in0=ssb[0:1, 0:6], scalar1=1.0 / GN_N,
                                    scalar2=None, op0=OP.mult)
            musq = spool.tile([1, 6], f32, tag=f"musq{b}", bufs=2, name=f"musq{b}_{g}")
            nc.vector.tensor_tensor(out=musq, in0=mu, in1=mu, op=OP.mult)
            var = spool.tile([1, 6], f32, tag=f"var{b}", bufs=2, name=f"var{b}_{g}")
            nc.vector.scalar_tensor_tensor(out=var, in0=ssb[0:1, 6:12],
                                           scalar=1.0 / GN_N, in1=musq,
                                           op0=OP.mult, op1=OP.subtract)
            lnv = spool.tile([1, 6], f32, tag=f"lnv{b}", bufs=2, name=f"lnv{b}_{g}")
            nc.scalar.activation(out=lnv, in_=var, func=AF.Ln, bias=eps_t, scale=1.0)
            rstd = spool.tile([1, 6], f32, tag=f"rstd{b}", bufs=2, name=f"rstd{b}_{g}")
            nc.scalar.activation(out=rstd, in_=lnv, func=AF.Exp, scale=-0.5)
            mu_b = spool.tile([64, 6], f32, tag=f"mu_b{b}", bufs=2, name=f"mu_b{b}_{g}")
            rstd_b = spool.tile([64, 6], f32, tag=f"rstd_b{b}", bufs=2,
                                name=f"rstd_b{b}_{g}")
            nc.sync.dma_start(out=mu_b, in_=bcast_ap(mu[0:1, :], 64))
            nc.sync.dma_start(out=rstd_b, in_=bcast_ap(rstd[0:1, :], 64))
            scale_all = spool.tile([64, 6], f32, tag=f"scl{b}", bufs=2,
                                   name=f"scl{b}_{g}")
            nc.vector.tensor_tensor(out=scale_all, in0=rstd_b,
                                    in1=gn_wT[:, 6 * g:6 * g + 6], op=OP.mult)
            bias_all = spool.tile([64, 6], f32, tag=f"bia{b}", bufs=2,
                                  name=f"bia{b}_{g}")
            nc.vector.scalar_tensor_tensor(out=bias_all, in0=mu_b, scalar=-1.0,
                                           in1=scale_all, op0=OP.mult, op1=OP.mult)
            nc.vector.tensor_tensor(out=bias_all, in0=bias_all,
                                    in1=gn_bT[:, 6 * g:6 * g + 6], op=OP.add)
            for j in range(6):
                h = 6 * g + j
                nc.vector.tensor_scalar(out=ctxf[b][0:64, h, 0:577],
                                        in0=ctxf[b][0:64, h, 0:577],
                                        scalar1=scale_all[:, j:j + 1],
                                        scalar2=bias_all[:, j:j + 1],
                                        op0=OP.mult, op1=OP.add)
            nc.sync.dma_start(
                out=bass.AP(tensor=scr[b].tensor,
                            offset=scr[b].offset + g * 6 * 64 * S,
                            ap=[[S, 64], [64 * S, 6], [1, S]]),
                in_=ctxf[b][0:64, 6 * g:6 * g + 6, 0:577])
        th.append(gn_apply)
        return th

    def p3_thunks(b):
        """Output projection per seq tile. Tiles 0-1 only need scratch rows
        from head group 0; tiles 2-4 need group 1 too. Returns (early, late)."""
        cT = big.tile([128, NK, 640], bf16, tag="ctxTT", bufs=1, name=f"ctxTT{b}")

        def o_thunk(t):
            def f():
                sz = 128 if t < NT - 1 else LAST
                cn = xpool.tile([128, D], bf16, tag="xb", name=f"cn{b}_{t}")
                if sz < 128:
                    nc.vector.memset(cn, 0.0)
                nc.gpsimd.dma_start(out=cn[0:sz, :], in_=scr[b][t * 128:t * 128 + sz, :])
                tp = sc_tile(f"tpc{b}_{t}", (128, 1536), bf16)
                for k in range(NK):
                    nc.tensor.transpose(tp[:, k * 128:(k + 1) * 128],
                                        cn[:, k * 128:(k + 1) * 128], ident)
                cTt = cT[:, 0:NK, t * 128:(t + 1) * 128]
                nc.vector.tensor_copy(cTt, tp[:, 0:768].rearrange("p (k c) -> p k c", k=NK))
                o_ps = sc_tile(f"o{b}_{t}")
                for k in range(NK):
                    nc.tensor.matmul(o_ps[:, 0:512], cT[:, k, t * 128:(t + 1) * 128],
                                     WoB[:, k, 0:512], start=(k == 0), stop=False,
                                     skip_group_check=True)
                    nc.tensor.matmul(o_ps[:, 512:768], cT[:, k, t * 128:(t + 1) * 128],
                                     WoB[:, k, 512:768], start=(k == 0), stop=False,
                                     skip_group_check=True)
                nc.tensor.matmul(o_ps[:, 0:512], onesrow, bob[0:1, 0:512],
                                 start=False, stop=True, skip_group_check=True)
                nc.tensor.matmul(o_ps[:, 512:768], onesrow, bob[0:1, 512:768],
                                 start=False, stop=True, skip_group_check=True)
                ot = xpool.tile([128, D], f32, tag="ot", name=f"ot{b}_{t}")
                nc.vector.tensor_copy(ot[0:sz, :], o_ps[0:sz, 0:768])
                nc.sync.dma_start(out=out[b, t * 128:t * 128 + sz, :], in_=ot[0:sz, :])
            return f
        return [o_thunk(0), o_thunk(1)], [o_thunk(2), o_thunk(3), o_thunk(4)]

    def drive(primary, fillers, hook=None):
        n, m = len(primary), len(fillers)
        fi = 0
        for i, p in enumerate(primary):
            p()
            if hook is not None:
                hook(i)
            target = (i + 1) * m // n
            while fi < target:
                fillers[fi]()
                fi += 1
        while fi < m:
            fillers[fi]()
            fi += 1

    # ---------------- emission ----------------
    p10 = p1_thunks(0)
    for t in p10[:NT]:
        t()
    emit_w_prep()
    for t in p10[NT:]:
        t()
    drive(attn_thunks(0), p1_thunks(1))

    # batch-0 tails + p3 fill attn(1); batch-1 group-0 tail fires mid-way
    tail0 = tail_thunks(0, 0) + tail_thunks(0, 1)
    p3e0, p3l0 = p3_thunks(0)
    tail1a = tail_thunks(1, 0)
    p3e1, p3l1 = p3_thunks(1)

    fired = [False]

    def hook(i):
        if i == 7 and not fired[0]:
            fired[0] = True
            for t in tail1a:
                t()

    drive(attn_thunks(1), tail0 + p3e0 + p3l0, hook=hook)
    if not fired[0]:
        for t in tail1a:
            t()
    for t in p3e1:
        t()
    for t in tail_thunks(1, 1):
        t()
    for t in p3l1:
        t()

    for p in (ps, drpool, spool, rpool, tpool, cpool, epool, xpool, wqk, big, sing):
        p.release()


_CACHE = {}
LAST_EXEC_NS = 0
LAST_TRACE = None


def _get_program(lam: float):
    key = round(float(lam), 8)
    if key not in _CACHE:
        _CACHE[key] = build_program(float(lam))
    return _CACHE[key]


def kernel(**inputs):
    x = np.ascontiguousarray(np.asarray(inputs["x"], dtype=np.float32))
    lam = float(np.asarray(inputs["lam"]))
    nc = _get_program(lam)
    names = ["Wq", "bq", "Wk", "bk", "Wv", "bv", "Wo", "bo", "gn_w", "gn_b"]
    shared = {n: np.ascontiguousarray(np.asarray(inputs[n], dtype=np.float32))
              for n in names}
    in_maps = []
    for c in range(N_CORES):
        m = dict(shared)
        m["x"] = x[c * BL:(c + 1) * BL]
        in_maps.append(m)
    res = bass_utils.run_bass_kernel_spmd(nc, in_maps, list(range(N_CORES)))
    global LAST_EXEC_NS, LAST_TRACE
    if getattr(res, "exec_time_ns", None):
        LAST_EXEC_NS = res.exec_time_ns
        LAST_TRACE = getattr(res, "instructions_and_trace", None)
    return np.concatenate([res.results[c]["out"] for c in range(N_CORES)], axis=0)


# revision 39
# speedup vs baseline: 1.3079x; 1.2462x over previous
"""Differential multi-head attention kernel for Trainium2 (8 NeuronCores).

Data-parallel over batch (16/8 = 2 per core). Per core, software-pipelined:

  init:  weights cast to bf16 once (Wq/Wk to DRAM scratch in a head-paired
         layout: head h's stationary cols are [q1|q2], so the dual-softmax
         score matmuls row-pack into PE array halves 0:64 / 64:128 and run
         concurrently). Wq loads go first so batch-0 Q-proj starts early.
  P1(b): x -> bf16 -> PE transpose -> xT; Q/K projections write Q12/K12
         (head h: side1 on partitions 0:64, side2 on 64:128); V -> vaug
         (ones col 64 makes the softmax denominators fall out of ctx MMs).
  P2(b): per head: row-packed score MMs into a merged [128,2,1024] psum,
         ONE exp per (h,kp) for both sides (ACT, bf16 out), ctx MMs
         accumulate [65, S]; ctx drained on DVE (row 64 = denominators,
         gathered into 32-aligned quadrants of den_all).
  tail(b), per 6-head group: reciprocals, -lam fold, bf16 broadcast,
         combines on DVE (stats via accum_out), per-group GroupNorm
         (rstd = exp(-0.5 ln(var+eps)) keeps ACT on one table set),
         apply, per-group scratch write.
  P3(b): read the bf16 scratch reinterpreted [S, D], PE transpose ->
         ctxTT, out = ctxTT.T @ Wo + bo. t-tiles 0:2 only need head
         group 0, so they start before group 1 finishes.

  Emission interleave: P1(b+1) fills the PE during P2(b); tail(0)/P3(0)
  and tail(1)-group0 fill DVE/PE during P2(1).
"""
import numpy as np

import concourse.bass as bass
import concourse.tile as tile
from concourse import mybir, bacc
from concourse import bass_utils
from concourse.masks import make_identity

f32 = mybir.dt.float32
bf16 = mybir.dt.bfloat16
AF = mybir.ActivationFunctionType
OP = mybir.AluOpType

B, S, D = 16, 577, 768
H, Dh = 12, 64
N_CORES = 8
BL = B // N_CORES
NK = D // 128              # 6 contraction chunks
NT = (S + 127) // 128      # 5 seq tiles
LAST = S - 4 * 128         # 65
SQ = 578
EPS = 1e-5
GN_N = float(Dh * S)
KW = [128, 128, 128, 128, LAST]


def bcast_ap(row_ap, nrows):
    """Partition-broadcast AP: repeat a single-partition row over nrows."""
    return bass.AP(tensor=row_ap.tensor, offset=row_ap.offset,
                   ap=[list(row_ap.ap[0]), [0, nrows]] + [list(x) for x in row_ap.ap[1:]])


def build_program(lam: float):
    nc = bacc.Bacc(trn_type="TRN2", target_bir_lowering=False, debug=False)

    x = nc.dram_tensor("x", [BL, S, D], f32, kind="ExternalInput").ap()
    Wq = nc.dram_tensor("Wq", [D, 2 * D], f32, kind="ExternalInput").ap()
    bq = nc.dram_tensor("bq", [2 * D], f32, kind="ExternalInput").ap()
    Wk = nc.dram_tensor("Wk", [D, 2 * D], f32, kind="ExternalInput").ap()
    bk = nc.dram_tensor("bk", [2 * D], f32, kind="ExternalInput").ap()
    Wv = nc.dram_tensor("Wv", [D, D], f32, kind="ExternalInput").ap()
    bv = nc.dram_tensor("bv", [D], f32, kind="ExternalInput").ap()
    Wo = nc.dram_tensor("Wo", [D, D], f32, kind="ExternalInput").ap()
    bo = nc.dram_tensor("bo", [D], f32, kind="ExternalInput").ap()
    gn_w = nc.dram_tensor("gn_w", [D], f32, kind="ExternalInput").ap()
    gn_b = nc.dram_tensor("gn_b", [D], f32, kind="ExternalInput").ap()
    out = nc.dram_tensor("out", [BL, S, D], f32, kind="ExternalOutput").ap()

    with tile.TileContext(nc) as tc:
        build_body(nc, tc, x, Wq, bq, Wk, bk, Wv, bv, Wo, bo, gn_w, gn_b, out, lam)
    nc.compile()
    return nc


def build_body(nc, tc, x, Wq, bq, Wk, bk, Wv, bv, Wo, bo, gn_w, gn_b, out, lam):
    sing = tc.alloc_tile_pool(name="sing", bufs=1)
    big = tc.alloc_tile_pool(name="big", bufs=1)
    wqk = tc.alloc_tile_pool(name="wqk", bufs=4)
    xpool = tc.alloc_tile_pool(name="xpool", bufs=2)
    epool = tc.alloc_tile_pool(name="epool", bufs=3)
    cpool = tc.alloc_tile_pool(name="cpool", bufs=13)
    tpool = tc.alloc_tile_pool(name="tpool", bufs=2)
    rpool = tc.alloc_tile_pool(name="rpool", bufs=2)
    spool = tc.alloc_tile_pool(name="spool", bufs=1)
    drpool = tc.alloc_tile_pool(name="drpool", bufs=1, space="DRAM")
    ps = tc.alloc_tile_pool(name="ps", bufs=1, space="PSUM")

    # "sc" slots (2 banks x 2 bufs): scores / projections / transposes / out
    def sc_tile(name, shape=(128, 768), dtype=f32):
        return ps.tile(list(shape), dtype, tag="sc", bufs=2, name=name,
                       padded_shape=None)

    def ctx_tile(name):
        return ps.tile([65, 640], f32, tag="ctx", bufs=2, name=name)

    # ---------------- singles ----------------
    ones64 = sing.tile([64, 1], f32, tag="ones64", name="ones64")
    nc.gpsimd.memset(ones64, 1.0)
    onesrow = sing.tile([1, 128], bf16, tag="onesrow", name="onesrow")
    nc.gpsimd.memset(onesrow, 1.0)
    eps_t = sing.tile([1, 1], f32, tag="eps_t", name="eps_t")
    nc.gpsimd.memset(eps_t, EPS)
    ident = sing.tile([128, 128], bf16, tag="ident", name="ident")
    make_identity(nc, ident)

    # head-paired biases: bqT12[p, h] = bq[64h+p] (p<64) | bq[D+64h+p-64]
    bqT12 = sing.tile([128, H], f32, tag="bqT12", name="bqT12")
    bkT12 = sing.tile([128, H], f32, tag="bkT12", name="bkT12")
    for bt, src in ((bqT12, bq), (bkT12, bk)):
        nc.sync.dma_start(out=bt[0:64, :],
                          in_=bass.AP(tensor=src.tensor, offset=src.offset,
                                      ap=[[1, 64], [64, H]]))
        nc.sync.dma_start(out=bt[64:128, :],
                          in_=bass.AP(tensor=src.tensor, offset=src.offset + D,
                                      ap=[[1, 64], [64, H]]))
    gn_wT = sing.tile([64, H], f32, tag="gn_wT", name="gn_wT")
    nc.sync.dma_start(out=gn_wT, in_=bass.AP(tensor=gn_w.tensor, offset=gn_w.offset,
                                             ap=[[1, 64], [64, H]]))
    gn_bT = sing.tile([64, H], f32, tag="gn_bT", name="gn_bT")
    nc.sync.dma_start(out=gn_bT, in_=bass.AP(tensor=gn_b.tensor, offset=gn_b.offset,
                                             ap=[[1, 64], [64, H]]))

    # bias rows -> bf16
    bvo16 = sing.tile([1, 2 * D], bf16, tag="bvo16", name="bvo16")
    for i, src in enumerate((bv, bo)):
        bt = xpool.tile([1, D], f32, tag="xn", name=f"bt{i}")
        nc.gpsimd.dma_start(out=bt,
                            in_=bass.AP(tensor=src.tensor, offset=src.offset,
                                        ap=[[D, 1], [1, D]]))
        nc.vector.tensor_copy(bvo16[0:1, i * D:(i + 1) * D], bt)
    bvb = bvo16[0:1, 0:D]
    bob = bvo16[0:1, D:2 * D]

    # Wv / Wo resident bf16; Wq / Wk -> bf16 DRAM scratch, head-paired
    # [k, p, h, side, 64]. Wq first (unblocks batch-0 Q-proj), Wo last.
    WvB = sing.tile([128, NK, D], bf16, tag="WvB", name="WvB")
    WoB = sing.tile([128, NK, D], bf16, tag="WoB", name="WoB")
    WqB = drpool.tile([NK, 128, H, 2, 64], bf16, tag="WqB", name="WqB")
    WkB = drpool.tile([NK, 128, H, 2, 64], bf16, tag="WkB", name="WkB")

    def emit_w_prep():
        qs = [nc.scalar, nc.sync]
        qi = 0

        def qk_prep(dstW, srcW, tagn):
            nonlocal qi
            for k in range(NK):
                for s in range(2):
                    wt = xpool.tile([128, D], f32, tag="ot", name=f"w{tagn}_{k}_{s}")
                    qs[qi % 2].dma_start(
                        out=wt, in_=srcW[k * 128:(k + 1) * 128, s * D:(s + 1) * D])
                    wc = xpool.tile([128, D], bf16, tag="xb", name=f"w{tagn}c_{k}_{s}")
                    nc.vector.tensor_copy(wc, wt)
                    nc.gpsimd.dma_start(
                        out=dstW[k][:, :, s, :],
                        in_=wc.rearrange("p (h c) -> p h c", h=H))
                    qi += 1

        def vo_prep(dstW, srcW, tagn):
            nonlocal qi
            for k in range(NK):
                wt = xpool.tile([128, D], f32, tag="ot", name=f"w{tagn}_{k}")
                qs[qi % 2].dma_start(out=wt, in_=srcW[k * 128:(k + 1) * 128, :])
                nc.vector.tensor_copy(dstW[:, k, :], wt)
                qi += 1

        qk_prep(WqB, Wq, "q")
        vo_prep(WvB, Wv, "v")
        qk_prep(WkB, Wk, "k")
        vo_prep(WoB, Wo, "o")

    # per-batch persistent tiles
    xT = [big.tile([128, NK, 640], bf16, tag=f"xT{b}", name=f"xT{b}") for b in range(BL)]
    Q12 = [big.tile([128, H, SQ], bf16, tag=f"Q12_{b}", name=f"Q12_{b}") for b in range(BL)]
    K12 = [big.tile([128, H, SQ], bf16, tag=f"K12_{b}", name=f"K12_{b}") for b in range(BL)]
    vaug = [big.tile([128, NT, H, 65], bf16, tag=f"vaug{b}", name=f"vaug{b}") for b in range(BL)]
    ctxf = [big.tile([65, H, SQ], bf16, tag=f"ctxf{b}", name=f"ctxf{b}") for b in range(BL)]
    # den_all quadrants (32-aligned for DVE partition-base rules):
    # head group g = h // 6, side s: row = 64*g + 32*s + (h % 6)
    den_all = [spool.tile([102, SQ], bf16, tag=f"den{b}", name=f"den{b}") for b in range(BL)]
    stats = [spool.tile([64, 2 * H], f32, tag=f"stats{b}", name=f"stats{b}") for b in range(BL)]
    csh2 = [[None] * H for _ in range(BL)]
    r16 = [spool.tile([102, SQ], bf16, tag=f"r16_{b}", name=f"r16_{b}") for b in range(BL)]
    scr = [drpool.tile([608, D], bf16, tag=f"scr{b}", name=f"scr{b}") for b in range(BL)]

    for b in range(BL):
        nc.gpsimd.memset(vaug[b][:, 0:NT - 1, :, 64:65], 1.0)
        nc.gpsimd.memset(vaug[b][0:LAST, NT - 1, :, 64:65], 1.0)

    # zero-fill scratch pad rows (577:608) so P3 transposes read finite data
    zpad = xpool.tile([128, D], bf16, tag="xb", name="zpad")
    nc.vector.memset(zpad, 0.0)
    for b in range(BL):
        nc.gpsimd.dma_start(out=scr[b][S:608, :], in_=zpad[0:608 - S, :])

    # ---------------- phase emitters ----------------
    def p1_thunks(b):
        th = []

        def x_thunk(t):
            def f():
                sz = 128 if t < NT - 1 else LAST
                xn = xpool.tile([128, D], f32, tag="xn", name=f"xn{b}_{t}")
                nc.gpsimd.dma_start(out=xn[0:sz, :], in_=x[b, t * 128:t * 128 + sz, :])
                xb = xpool.tile([128, D], bf16, tag="xb", name=f"xb{b}_{t}")
                if sz < 128:
                    nc.vector.memset(xb, 0.0)
                nc.vector.tensor_copy(xb[0:sz, :], xn[0:sz, :])
                tp = sc_tile(f"tpx{b}_{t}", (128, 1536), bf16)
                for k in range(NK):
                    nc.tensor.transpose(tp[:, k * 128:(k + 1) * 128],
                                        xb[:, k * 128:(k + 1) * 128], ident)
                nc.vector.tensor_copy(
                    xT[b][:, 0:NK, t * 128:(t + 1) * 128],
                    tp[:, 0:768].rearrange("p (k c) -> p k c", k=NK))
            return f

        def qk_thunk(h, WB, dstT, biasT, nm):
            def f():
                wq = wqk.tile([128, NK, 2, 64], bf16, tag="wqk", name=f"w{nm}{b}_{h}")
                nc.gpsimd.dma_start(
                    out=wq,
                    in_=bass.AP(tensor=WB.tensor, offset=WB.offset + h * 128,
                                ap=[[H * 128, 128], [128 * H * 128, NK], [1, 128]]))
                q_ps = sc_tile(f"ps{nm}{b}_{h}")
                for k in range(NK):
                    nc.tensor.matmul(q_ps[:, 0:512], wq[:, k], xT[b][:, k, 0:512],
                                     start=(k == 0), stop=(k == NK - 1),
                                     skip_group_check=True)
                    nc.tensor.matmul(q_ps[:, 512:577], wq[:, k], xT[b][:, k, 512:577],
                                     start=(k == 0), stop=(k == NK - 1),
                                     skip_group_check=True)
                nc.vector.tensor_scalar(out=dstT[b][:, h, 0:577], in0=q_ps[:, 0:577],
                                        scalar1=biasT[:, h:h + 1], scalar2=None,
                                        op0=OP.add)
            return f

        def v_thunk(t):
            def f():
                sz = 128 if t < NT - 1 else LAST
                v_ps = sc_tile(f"psv{b}_{t}")
                for k in range(NK):
                    nc.tensor.matmul(v_ps[:, 0:512], xT[b][:, k, t * 128:(t + 1) * 128],
                                     WvB[:, k, 0:512], start=(k == 0), stop=False,
                                     skip_group_check=True)
                    nc.tensor.matmul(v_ps[:, 512:768], xT[b][:, k, t * 128:(t + 1) * 128],
                                     WvB[:, k, 512:768], start=(k == 0), stop=False,
                                     skip_group_check=True)
                nc.tensor.matmul(v_ps[:, 0:512], onesrow, bvb[0:1, 0:512],
                                 start=False, stop=True, skip_group_check=True)
                nc.tensor.matmul(v_ps[:, 512:768], onesrow, bvb[0:1, 512:768],
                                 start=False, stop=True, skip_group_check=True)
                nc.vector.tensor_copy(vaug[b][0:sz, t, 0:6, 0:64],
                                      v_ps[0:sz, 0:384].rearrange("p (h d) -> p h d", h=6))
                nc.vector.tensor_copy(vaug[b][0:sz, t, 6:12, 0:64],
                                      v_ps[0:sz, 384:768].rearrange("p (h d) -> p h d", h=6))
            return f

        for t in range(NT):
            th.append(x_thunk(t))
        for h in range(H):
            th.append(qk_thunk(h, WqB, Q12, bqT12, "q"))
        for h in range(H):
            th.append(qk_thunk(h, WkB, K12, bkT12, "k"))
        for t in range(NT):
            th.append(v_thunk(t))
        return th

    def attn_thunks(b):
        th = []

        def head_thunk(h):
            def f():
                c1 = ctx_tile(f"c1_{b}_{h}")
                c2 = ctx_tile(f"c2_{b}_{h}")
                for kp in range(NT):
                    kw = KW[kp]
                    ksl = slice(kp * 128, kp * 128 + kw)
                    e = epool.tile([128, 2, SQ], bf16, tag="e", name=f"e{b}_{h}_{kp}")
                    ss = []
                    for side in range(2):
                        off = side * 64
                        s_ps = sc_tile(f"s{side}_{b}_{h}_{kp}")
                        nc.tensor.matmul(s_ps[0:kw, 0:512],
                                         K12[b][off:off + 64, h, ksl],
                                         Q12[b][off:off + 64, h, 0:512],
                                         start=True, stop=True, skip_group_check=True)
                        nc.tensor.matmul(s_ps[0:kw, 512:577],
                                         K12[b][off:off + 64, h, ksl],
                                         Q12[b][off:off + 64, h, 512:577],
                                         start=True, stop=True, skip_group_check=True)
                        ss.append(s_ps)
                    for side in range(2):
                        nc.scalar.activation(out=e[0:kw, side, 0:577],
                                             in_=ss[side][0:kw, 0:577],
                                             func=AF.Exp, scale=0.125)
                    for side, c in ((0, c1), (1, c2)):
                        nc.tensor.matmul(c[:, 0:512], vaug[b][0:kw, kp, h, :],
                                         e[0:kw, side, 0:512],
                                         start=(kp == 0), stop=False,
                                         skip_group_check=True)
                        nc.tensor.matmul(c[:, 512:577], vaug[b][0:kw, kp, h, :],
                                         e[0:kw, side, 512:577],
                                         start=(kp == 0), stop=(kp == NT - 1),
                                         skip_group_check=True)
                nc.vector.tensor_copy(ctxf[b][0:65, h, 0:577], c1[0:65, 0:577])
                ch2 = cpool.tile([65, SQ], bf16, tag="csh2", name=f"csh2_{b}_{h}")
                csh2[b][h] = ch2
                nc.vector.tensor_copy(ch2[0:65, 0:577], c2[0:65, 0:577])
                g = h // 6
                if h % 3 == 2:
                    # batched side-1 denominator gather for heads h-2..h
                    r0 = 64 * g + (h % 6) - 2
                    nc.sync.dma_start(out=den_all[b][r0:r0 + 3, 0:577],
                                      in_=ctxf[b][64:65, h - 2:h + 1, 0:577])
                nc.gpsimd.dma_start(
                    out=den_all[b][64 * g + 32 + (h % 6):64 * g + 33 + (h % 6), 0:577],
                    in_=ch2[64:65, 0:577])
            return f

        for h in range(H):
            th.append(head_thunk(h))
        return th

    def tail_thunks(b, g):
        """Tail for head group g (heads 6g..6g+5): recip, combines, GN, apply,
        scratch write. Returns a thunk list."""
        th = []
        q0 = 64 * g

        def recip():
            r_all = spool.tile([102, SQ], f32, tag=f"rall{b}", bufs=2,
                               name=f"rall{b}_{g}")
            nc.vector.reciprocal(out=r_all[q0:q0 + 6, 0:577],
                                 in_=den_all[b][q0:q0 + 6, 0:577])
            nc.vector.reciprocal(out=r_all[q0 + 32:q0 + 38, 0:577],
                                 in_=den_all[b][q0 + 32:q0 + 38, 0:577])
            nc.vector.tensor_copy(r16[b][q0:q0 + 6, 0:577], r_all[q0:q0 + 6, 0:577])
            nc.vector.tensor_scalar(out=r16[b][q0 + 32:q0 + 38, 0:577],
                                    in0=r_all[q0 + 32:q0 + 38, 0:577],
                                    scalar1=-lam, scalar2=None, op0=OP.mult)
        th.append(recip)

        def combine(h):
            def f():
                rb = rpool.tile([64, 2, SQ], bf16, tag="rb", name=f"rb{b}_{h}")
                for side in range(2):
                    row = q0 + 32 * side + (h % 6)
                    nc.sync.dma_start(out=rb[:, side, 0:577],
                                      in_=bcast_ap(r16[b][row:row + 1, 0:577], 64))
                tmp = tpool.tile([64, SQ], bf16, tag="tmp", name=f"tmp{b}_{h}")
                ch = ctxf[b][0:64, h, 0:577]
                ch2 = csh2[b][h]
                nc.vector.tensor_tensor(out=tmp[:, 0:577], in0=ch, in1=rb[:, 0, 0:577],
                                        op=OP.mult)
                nc.vector.tensor_tensor(out=ch2[0:64, 0:577], in0=ch2[0:64, 0:577],
                                        in1=rb[:, 1, 0:577], op=OP.mult)
                nc.vector.scalar_tensor_tensor(out=ch, in0=tmp[:, 0:577], scalar=1.0,
                                               in1=ch2[0:64, 0:577],
                                               op0=OP.mult, op1=OP.add,
                                               accum_out=stats[b][:, h:h + 1])
                nc.vector.scalar_tensor_tensor(out=tmp[:, 0:577], in0=ch, scalar=1.0,
                                               in1=ch, op0=OP.mult, op1=OP.mult,
                                               accum_out=stats[b][:, H + h:H + h + 1])
            return f
        for h in range(6 * g, 6 * g + 6):
            th.append(combine(h))

        def gn_apply():
            sps = ctx_tile(f"gn{b}_{g}")
            stats_g = stats[b].rearrange("p (a c) -> p a c", a=2)[:, :, 6 * g:6 * g + 6]
            nc.tensor.matmul(sps[0:1, 0:12], ones64, stats_g, start=True, stop=True,
                             skip_group_check=True)
            ssb = spool.tile([1, 12], f32, tag=f"ssb{b}", bufs=2, name=f"ssb{b}_{g}")
            nc.vector.tensor_copy(ssb, sps[0:1, 0:12])
            mu = spool.tile([1, 6], f32, tag=f"mu{b}", bufs=2, name=f"mu{b}_{g}")
            nc.vector.tensor_scalar(out=mu, in0=ssb[0:1, 0:6], scalar1=1.0 / GN_N,
                                    scalar2=None, op0=OP.mult)
            musq = spool.tile([1, 6], f32, tag=f"musq{b}", bufs=2, name=f"musq{b}_{g}")
            nc.vector.tensor_tensor(out=musq, in0=mu, in1=mu, op=OP.mult)
            var = spool.tile([1, 6], f32, tag=f"var{b}", bufs=2, name=f"var{b}_{g}")
            nc.vector.scalar_tensor_tensor(out=var, in0=ssb[0:1, 6:12],
                                           scalar=1.0 / GN_N, in1=musq,
                                           op0=OP.mult, op1=OP.subtract)
            lnv = spool.tile([1, 6], f32, tag=f"lnv{b}", bufs=2, name=f"lnv{b}_{g}")
            nc.scalar.activation(out=lnv, in_=var, func=AF.Ln, bias=eps_t, scale=1.0)
            rstd = spool.tile([1, 6], f32, tag=f"rstd{b}", bufs=2, name=f"rstd{b}_{g}")
            nc.scalar.activation(out=rstd, in_=lnv, func=AF.Exp, scale=-0.5)
            mu_b = spool.tile([64, 6], f32, tag=f"mu_b{b}", bufs=2, name=f"mu_b{b}_{g}")
            rstd_b = spool.tile([64, 6], f32, tag=f"rstd_b{b}", bufs=2,
                                name=f"rstd_b{b}_{g}")
            nc.sync.dma_start(out=mu_b, in_=bcast_ap(mu[0:1, :], 64))
            nc.sync.dma_start(out=rstd_b, in_=bcast_ap(rstd[0:1, :], 64))
            scale_all = spool.tile([64, 6], f32, tag=f"scl{b}", bufs=2,
                                   name=f"scl{b}_{g}")
            nc.vector.tensor_tensor(out=scale_all, in0=rstd_b,
                                    in1=gn_wT[:, 6 * g:6 * g + 6], op=OP.mult)
            bias_all = spool.tile([64, 6], f32, tag=f"bia{b}", bufs=2,
                                  name=f"bia{b}_{g}")
            nc.vector.scalar_tensor_tensor(out=bias_all, in0=mu_b, scalar=-1.0,
                                           in1=scale_all, op0=OP.mult, op1=OP.mult)
            nc.vector.tensor_tensor(out=bias_all, in0=bias_all,
                                    in1=gn_bT[:, 6 * g:6 * g + 6], op=OP.add)
            for j in range(6):
                h = 6 * g + j
                nc.vector.tensor_scalar(out=ctxf[b][0:64, h, 0:577],
                                        in0=ctxf[b][0:64, h, 0:577],
                                        scalar1=scale_all[:, j:j + 1],
                                        scalar2=bias_all[:, j:j + 1],
                                        op0=OP.mult, op1=OP.add)
            nc.sync.dma_start(
                out=bass.AP(tensor=scr[b].tensor,
                            offset=scr[b].offset + g * 6 * 64 * S,
                            ap=[[S, 64], [64 * S, 6], [1, S]]),
                in_=ctxf[b][0:64, 6 * g:6 * g + 6, 0:577])
        th.append(gn_apply)
        return th

    def p3_thunks(b):
        """Output projection per seq tile. Tiles 0-1 only need scratch rows
        from head group 0; tiles 2-4 need group 1 too. Returns (early, late)."""
        cT = big.tile([128, NK, 640], bf16, tag="ctxTT", bufs=1, name=f"ctxTT{b}")

        def o_thunk(t):
            def f():
                sz = 128 if t < NT - 1 else LAST
                cn = xpool.tile([128, D], bf16, tag="xb", name=f"cn{b}_{t}")
                if sz < 128:
                    nc.vector.memset(cn, 0.0)
                nc.gpsimd.dma_start(out=cn[0:sz, :], in_=scr[b][t * 128:t * 128 + sz, :])
                tp = sc_tile(f"tpc{b}_{t}", (128, 1536), bf16)
                for k in range(NK):
                    nc.tensor.transpose(tp[:, k * 128:(k + 1) * 128],
                                        cn[:, k * 128:(k + 1) * 128], ident)
                cTt = cT[:, 0:NK, t * 128:(t + 1) * 128]
                nc.vector.tensor_copy(cTt, tp[:, 0:768].rearrange("p (k c) -> p k c", k=NK))
                o_ps = sc_tile(f"o{b}_{t}")
                for k in range(NK):
                    nc.tensor.matmul(o_ps[:, 0:512], cT[:, k, t * 128:(t + 1) * 128],
                                     WoB[:, k, 0:512], start=(k == 0), stop=False,
                                     skip_group_check=True)
                    nc.tensor.matmul(o_ps[:, 512:768], cT[:, k, t * 128:(t + 1) * 128],
                                     WoB[:, k, 512:768], start=(k == 0), stop=False,
                                     skip_group_check=True)
                nc.tensor.matmul(o_ps[:, 0:512], onesrow, bob[0:1, 0:512],
                                 start=False, stop=True, skip_group_check=True)
                nc.tensor.matmul(o_ps[:, 512:768], onesrow, bob[0:1, 512:768],
                                 start=False, stop=True, skip_group_check=True)
                ot = xpool.tile([128, D], f32, tag="ot", name=f"ot{b}_{t}")
                nc.vector.tensor_copy(ot[0:sz, :], o_ps[0:sz, 0:768])
                nc.sync.dma_start(out=out[b, t * 128:t * 128 + sz, :], in_=ot[0:sz, :])
            return f
        return [o_thunk(0), o_thunk(1)], [o_thunk(2), o_thunk(3), o_thunk(4)]

    def drive(primary, fillers, hook=None):
        n, m = len(primary), len(fillers)
        fi = 0
        for i, p in enumerate(primary):
            p()
            if hook is not None:
                hook(i)
            target = (i + 1) * m // n
            while fi < target:
                fillers[fi]()
                fi += 1
        while fi < m:
            fillers[fi]()
            fi += 1

    # ---------------- emission ----------------
    p10 = p1_thunks(0)
    for t in p10[:NT]:
        t()
    emit_w_prep()
    for t in p10[NT:]:
        t()
    drive(attn_thunks(0), p1_thunks(1))

    # batch-0 tails + p3 fill attn(1); batch-1 group-0 tail fires mid-way
    tail0 = tail_thunks(0, 0) + tail_thunks(0, 1)
    p3e0, p3l0 = p3_thunks(0)
    tail1a = tail_thunks(1, 0)
    p3e1, p3l1 = p3_thunks(1)

    fired = [False]

    def hook(i):
        if i == 7 and not fired[0]:
            fired[0] = True
            for t in tail1a:
                t()

    drive(attn_thunks(1), tail0 + p3e0 + p3l0, hook=hook)
    if not fired[0]:
        for t in tail1a:
            t()
    for t in p3e1:
        t()
    for t in tail_thunks(1, 1):
        t()
    for t in p3l1:
        t()

    for p in (ps, drpool, spool, rpool, tpool, cpool, epool, xpool, wqk, big, sing):
        p.release()


_CACHE = {}
LAST_EXEC_NS = 0
LAST_TRACE = None


def _get_program(lam: float):
    key = round(float(lam), 8)
    if key not in _CACHE:
        _CACHE[key] = build_program(float(lam))
    return _CACHE[key]


def kernel(**inputs):
    x = np.ascontiguousarray(np.asarray(inputs["x"], dtype=np.float32))
    lam = float(np.asarray(inputs["lam"]))
    nc = _get_program(lam)
    names = ["Wq", "bq", "Wk", "bk", "Wv", "bv", "Wo", "bo", "gn_w", "gn_b"]
    shared = {n: np.ascontiguousarray(np.asarray(inputs[n], dtype=np.float32))
              for n in names}
    in_maps = []
    for c in range(N_CORES):
        m = dict(shared)
        m["x"] = x[c * BL:(c + 1) * BL]
        in_maps.append(m)
    res = bass_utils.run_bass_kernel_spmd(nc, in_maps, list(range(N_CORES)))
    global LAST_EXEC_NS, LAST_TRACE
    if getattr(res, "exec_time_ns", None):
        LAST_EXEC_NS = res.exec_time_ns
        LAST_TRACE = getattr(res, "instructions_and_trace", None)
    return np.concatenate([res.results[c]["out"] for c in range(N_CORES)], axis=0)


# revision 40
# speedup vs baseline: 1.4213x; 1.0867x over previous
"""Differential multi-head attention kernel for Trainium2 (8 NeuronCores).

Data-parallel over batch (16/8 = 2 per core). Per core, software-pipelined:

  init:  weights cast to bf16 once (Wq/Wk to DRAM scratch in a head-paired
         layout: head h's stationary cols are [q1|q2], so the dual-softmax
         score matmuls row-pack into PE array halves 0:64 / 64:128 and run
         concurrently). Wq loads go first so batch-0 Q-proj starts early.
  P1(b): x -> bf16 -> PE transpose -> xT; Q/K projections write Q12/K12
         (head h: side1 on partitions 0:64, side2 on 64:128); V -> vaug
         (ones col 64 makes the softmax denominators fall out of ctx MMs).
  P2(b): per head: row-packed score MMs into a merged [128,2,1024] psum,
         ONE exp per (h,kp) for both sides (ACT, bf16 out), ctx MMs
         accumulate [65, S]; ctx drained on DVE (row 64 = denominators,
         gathered into 32-aligned quadrants of den_all).
  tail(b), per 6-head group: reciprocals, -lam fold, bf16 broadcast,
         combines on DVE (stats via accum_out), per-group GroupNorm
         (rstd = exp(-0.5 ln(var+eps)) keeps ACT on one table set),
         apply, per-group scratch write.
  P3(b): read the bf16 scratch reinterpreted [S, D], PE transpose ->
         ctxTT, out = ctxTT.T @ Wo + bo. t-tiles 0:2 only need head
         group 0, so they start before group 1 finishes.

  Emission interleave: P1(b+1) fills the PE during P2(b); tail(0)/P3(0)
  and tail(1)-group0 fill DVE/PE during P2(1).
"""
import numpy as np

import concourse.bass as bass
import concourse.tile as tile
from concourse import mybir, bacc
from concourse import bass_utils
from concourse.masks import make_identity

f32 = mybir.dt.float32
bf16 = mybir.dt.bfloat16
AF = mybir.ActivationFunctionType
OP = mybir.AluOpType

B, S, D = 16, 577, 768
H, Dh = 12, 64
N_CORES = 8
BL = B // N_CORES
NK = D // 128              # 6 contraction chunks
NT = (S + 127) // 128      # 5 seq tiles
LAST = S - 4 * 128         # 65
SQ = 578
EPS = 1e-5
GN_N = float(Dh * S)
KW = [128, 128, 128, 128, LAST]


def bcast_ap(row_ap, nrows):
    """Partition-broadcast AP: repeat a single-partition row over nrows."""
    return bass.AP(tensor=row_ap.tensor, offset=row_ap.offset,
                   ap=[list(row_ap.ap[0]), [0, nrows]] + [list(x) for x in row_ap.ap[1:]])


def build_program(lam: float):
    nc = bacc.Bacc(trn_type="TRN2", target_bir_lowering=False, debug=False)

    x = nc.dram_tensor("x", [BL, S, D], f32, kind="ExternalInput").ap()
    Wq = nc.dram_tensor("Wq", [D, 2 * D], f32, kind="ExternalInput").ap()
    bq = nc.dram_tensor("bq", [2 * D], f32, kind="ExternalInput").ap()
    Wk = nc.dram_tensor("Wk", [D, 2 * D], f32, kind="ExternalInput").ap()
    bk = nc.dram_tensor("bk", [2 * D], f32, kind="ExternalInput").ap()
    Wv = nc.dram_tensor("Wv", [D, D], f32, kind="ExternalInput").ap()
    bv = nc.dram_tensor("bv", [D], f32, kind="ExternalInput").ap()
    Wo = nc.dram_tensor("Wo", [D, D], f32, kind="ExternalInput").ap()
    bo = nc.dram_tensor("bo", [D], f32, kind="ExternalInput").ap()
    gn_w = nc.dram_tensor("gn_w", [D], f32, kind="ExternalInput").ap()
    gn_b = nc.dram_tensor("gn_b", [D], f32, kind="ExternalInput").ap()
    out = nc.dram_tensor("out", [BL, S, D], f32, kind="ExternalOutput").ap()

    with tile.TileContext(nc) as tc:
        build_body(nc, tc, x, Wq, bq, Wk, bk, Wv, bv, Wo, bo, gn_w, gn_b, out, lam)
    nc.compile()
    return nc


def build_body(nc, tc, x, Wq, bq, Wk, bk, Wv, bv, Wo, bo, gn_w, gn_b, out, lam):
    sing = tc.alloc_tile_pool(name="sing", bufs=1)
    big = tc.alloc_tile_pool(name="big", bufs=1)
    wqk = tc.alloc_tile_pool(name="wqk", bufs=4)
    xpool = tc.alloc_tile_pool(name="xpool", bufs=2)
    epool = tc.alloc_tile_pool(name="epool", bufs=3)
    cpool = tc.alloc_tile_pool(name="cpool", bufs=13)
    tpool = tc.alloc_tile_pool(name="tpool", bufs=2)
    rpool = tc.alloc_tile_pool(name="rpool", bufs=2)
    spool = tc.alloc_tile_pool(name="spool", bufs=1)
    drpool = tc.alloc_tile_pool(name="drpool", bufs=1, space="DRAM")
    ps = tc.alloc_tile_pool(name="ps", bufs=1, space="PSUM")

    # "sc" slots (2 banks x 2 bufs): exclusively the score matmuls, so the
    # exp cadence never stalls on interleaved projection work.
    def sc_tile(name, shape=(128, 768), dtype=f32):
        return ps.tile(list(shape), dtype, tag="sc", bufs=2, name=name,
                       padded_shape=None)

    # ctx accumulators + everything else (projections, V, transposes, out,
    # GN reduce) share the other 2x2-bank ring.
    def ctx_tile(name):
        return ps.tile([65, 640], f32, tag="ctx", bufs=2, name=name)

    def aux_tile(name, shape=(128, 768), dtype=f32):
        return ps.tile(list(shape), dtype, tag="ctx", bufs=2, name=name)

    # ---------------- singles ----------------
    ones64 = sing.tile([64, 1], f32, tag="ones64", name="ones64")
    nc.gpsimd.memset(ones64, 1.0)
    onesrow = sing.tile([1, 128], bf16, tag="onesrow", name="onesrow")
    nc.gpsimd.memset(onesrow, 1.0)
    eps_t = sing.tile([1, 1], f32, tag="eps_t", name="eps_t")
    nc.gpsimd.memset(eps_t, EPS)
    ident = sing.tile([128, 128], bf16, tag="ident", name="ident")
    make_identity(nc, ident)

    # head-paired biases: bqT12[p, h] = bq[64h+p] (p<64) | bq[D+64h+p-64]
    bqT12 = sing.tile([128, H], f32, tag="bqT12", name="bqT12")
    bkT12 = sing.tile([128, H], f32, tag="bkT12", name="bkT12")
    for bt, src in ((bqT12, bq), (bkT12, bk)):
        nc.sync.dma_start(out=bt[0:64, :],
                          in_=bass.AP(tensor=src.tensor, offset=src.offset,
                                      ap=[[1, 64], [64, H]]))
        nc.sync.dma_start(out=bt[64:128, :],
                          in_=bass.AP(tensor=src.tensor, offset=src.offset + D,
                                      ap=[[1, 64], [64, H]]))
    gn_wT = sing.tile([64, H], f32, tag="gn_wT", name="gn_wT")
    nc.sync.dma_start(out=gn_wT, in_=bass.AP(tensor=gn_w.tensor, offset=gn_w.offset,
                                             ap=[[1, 64], [64, H]]))
    gn_bT = sing.tile([64, H], f32, tag="gn_bT", name="gn_bT")
    nc.sync.dma_start(out=gn_bT, in_=bass.AP(tensor=gn_b.tensor, offset=gn_b.offset,
                                             ap=[[1, 64], [64, H]]))

    # bias rows -> bf16
    bvo16 = sing.tile([1, 2 * D], bf16, tag="bvo16", name="bvo16")
    for i, src in enumerate((bv, bo)):
        bt = xpool.tile([1, D], f32, tag="xn", name=f"bt{i}")
        nc.gpsimd.dma_start(out=bt,
                            in_=bass.AP(tensor=src.tensor, offset=src.offset,
                                        ap=[[D, 1], [1, D]]))
        nc.vector.tensor_copy(bvo16[0:1, i * D:(i + 1) * D], bt)
    bvb = bvo16[0:1, 0:D]
    bob = bvo16[0:1, D:2 * D]

    # Wv / Wo resident bf16; Wq / Wk -> bf16 DRAM scratch, head-paired
    # [k, p, h, side, 64]. Wq first (unblocks batch-0 Q-proj), Wo last.
    WvB = sing.tile([128, NK, D], bf16, tag="WvB", name="WvB")
    WoB = sing.tile([128, NK, D], bf16, tag="WoB", name="WoB")
    WqB = drpool.tile([NK, 128, H, 2, 64], bf16, tag="WqB", name="WqB")
    WkB = drpool.tile([NK, 128, H, 2, 64], bf16, tag="WkB", name="WkB")

    def emit_w_prep():
        qs = [nc.scalar, nc.sync]
        qi = 0

        def qk_prep(dstW, srcW, tagn):
            nonlocal qi
            for k in range(NK):
                for s in range(2):
                    wt = xpool.tile([128, D], f32, tag="ot", name=f"w{tagn}_{k}_{s}")
                    qs[qi % 2].dma_start(
                        out=wt, in_=srcW[k * 128:(k + 1) * 128, s * D:(s + 1) * D])
                    wc = xpool.tile([128, D], bf16, tag="xb", name=f"w{tagn}c_{k}_{s}")
                    nc.vector.tensor_copy(wc, wt)
                    nc.gpsimd.dma_start(
                        out=dstW[k][:, :, s, :],
                        in_=wc.rearrange("p (h c) -> p h c", h=H))
                    qi += 1

        def vo_prep(dstW, srcW, tagn):
            nonlocal qi
            for k in range(NK):
                wt = xpool.tile([128, D], f32, tag="ot", name=f"w{tagn}_{k}")
                qs[qi % 2].dma_start(out=wt, in_=srcW[k * 128:(k + 1) * 128, :])
                nc.vector.tensor_copy(dstW[:, k, :], wt)
                qi += 1

        return (lambda: qk_prep(WqB, Wq, "q"), lambda: vo_prep(WvB, Wv, "v"),
                lambda: qk_prep(WkB, Wk, "k"), lambda: vo_prep(WoB, Wo, "o"))

    # per-batch persistent tiles
    xT = [big.tile([128, NK, 640], bf16, tag=f"xT{b}", name=f"xT{b}") for b in range(BL)]
    Q12 = [big.tile([128, H, SQ], bf16, tag=f"Q12_{b}", name=f"Q12_{b}") for b in range(BL)]
    K12 = [big.tile([128, H, SQ], bf16, tag=f"K12_{b}", name=f"K12_{b}") for b in range(BL)]
    vaug = [big.tile([128, NT, H, 65], bf16, tag=f"vaug{b}", name=f"vaug{b}") for b in range(BL)]
    ctxf = [big.tile([65, H, SQ], bf16, tag=f"ctxf{b}", name=f"ctxf{b}") for b in range(BL)]
    # den_all quadrants (32-aligned for DVE partition-base rules):
    # head group g = h // 6, side s: row = 64*g + 32*s + (h % 6)
    den_all = [spool.tile([102, SQ], bf16, tag=f"den{b}", name=f"den{b}") for b in range(BL)]
    stats = [spool.tile([64, 2 * H], f32, tag=f"stats{b}", name=f"stats{b}") for b in range(BL)]
    csh2 = [[None] * H for _ in range(BL)]
    r16 = [spool.tile([102, SQ], bf16, tag=f"r16_{b}", name=f"r16_{b}") for b in range(BL)]
    scr = [drpool.tile([608, D], bf16, tag=f"scr{b}", name=f"scr{b}") for b in range(BL)]

    for b in range(BL):
        nc.gpsimd.memset(vaug[b][:, 0:NT - 1, :, 64:65], 1.0)
        nc.gpsimd.memset(vaug[b][0:LAST, NT - 1, :, 64:65], 1.0)

    # zero-fill scratch pad rows (577:608) so P3 transposes read finite data
    zpad = xpool.tile([128, D], bf16, tag="xb", name="zpad")
    nc.vector.memset(zpad, 0.0)
    for b in range(BL):
        nc.gpsimd.dma_start(out=scr[b][S:608, :], in_=zpad[0:608 - S, :])

    # ---------------- phase emitters ----------------
    def p1_thunks(b):
        th = []

        def x_thunk(t):
            def f():
                sz = 128 if t < NT - 1 else LAST
                xn = xpool.tile([128, D], f32, tag="xn", name=f"xn{b}_{t}")
                nc.gpsimd.dma_start(out=xn[0:sz, :], in_=x[b, t * 128:t * 128 + sz, :])
                xb = xpool.tile([128, D], bf16, tag="xb", name=f"xb{b}_{t}")
                if sz < 128:
                    nc.vector.memset(xb, 0.0)
                nc.vector.tensor_copy(xb[0:sz, :], xn[0:sz, :])
                tp = aux_tile(f"tpx{b}_{t}", (128, 1536), bf16)
                for k in range(NK):
                    nc.tensor.transpose(tp[:, k * 128:(k + 1) * 128],
                                        xb[:, k * 128:(k + 1) * 128], ident)
                nc.vector.tensor_copy(
                    xT[b][:, 0:NK, t * 128:(t + 1) * 128],
                    tp[:, 0:768].rearrange("p (k c) -> p k c", k=NK))
            return f

        def qk_thunk(h, WB, dstT, biasT, nm):
            def f():
                wq = wqk.tile([128, NK, 2, 64], bf16, tag="wqk", name=f"w{nm}{b}_{h}")
                nc.gpsimd.dma_start(
                    out=wq,
                    in_=bass.AP(tensor=WB.tensor, offset=WB.offset + h * 128,
                                ap=[[H * 128, 128], [128 * H * 128, NK], [1, 128]]))
                q_ps = aux_tile(f"ps{nm}{b}_{h}")
                for k in range(NK):
                    nc.tensor.matmul(q_ps[:, 0:512], wq[:, k], xT[b][:, k, 0:512],
                                     start=(k == 0), stop=(k == NK - 1),
                                     skip_group_check=True)
                    nc.tensor.matmul(q_ps[:, 512:577], wq[:, k], xT[b][:, k, 512:577],
                                     start=(k == 0), stop=(k == NK - 1),
                                     skip_group_check=True)
                nc.vector.tensor_scalar(out=dstT[b][:, h, 0:577], in0=q_ps[:, 0:577],
                                        scalar1=biasT[:, h:h + 1], scalar2=None,
                                        op0=OP.add)
            return f

        def v_thunk(t):
            def f():
                sz = 128 if t < NT - 1 else LAST
                v_ps = aux_tile(f"psv{b}_{t}")
                for k in range(NK):
                    nc.tensor.matmul(v_ps[:, 0:512], xT[b][:, k, t * 128:(t + 1) * 128],
                                     WvB[:, k, 0:512], start=(k == 0), stop=False,
                                     skip_group_check=True)
                    nc.tensor.matmul(v_ps[:, 512:768], xT[b][:, k, t * 128:(t + 1) * 128],
                                     WvB[:, k, 512:768], start=(k == 0), stop=False,
                                     skip_group_check=True)
                nc.tensor.matmul(v_ps[:, 0:512], onesrow, bvb[0:1, 0:512],
                                 start=False, stop=True, skip_group_check=True)
                nc.tensor.matmul(v_ps[:, 512:768], onesrow, bvb[0:1, 512:768],
                                 start=False, stop=True, skip_group_check=True)
                nc.vector.tensor_copy(vaug[b][0:sz, t, 0:6, 0:64],
                                      v_ps[0:sz, 0:384].rearrange("p (h d) -> p h d", h=6))
                nc.vector.tensor_copy(vaug[b][0:sz, t, 6:12, 0:64],
                                      v_ps[0:sz, 384:768].rearrange("p (h d) -> p h d", h=6))
            return f

        xs = [x_thunk(t) for t in range(NT)]
        qs_ = [qk_thunk(h, WqB, Q12, bqT12, "q") for h in range(H)]
        ks_ = [qk_thunk(h, WkB, K12, bkT12, "k") for h in range(H)]
        vs_ = [v_thunk(t) for t in range(NT)]
        return xs, qs_, ks_, vs_

    def attn_thunks(b):
        th = []

        def head_thunk(h):
            def f():
                c1 = ctx_tile(f"c1_{b}_{h}")
                c2 = ctx_tile(f"c2_{b}_{h}")
                for kp in range(NT):
                    kw = KW[kp]
                    ksl = slice(kp * 128, kp * 128 + kw)
                    e = epool.tile([128, 2, SQ], bf16, tag="e", name=f"e{b}_{h}_{kp}")
                    ss = []
                    for side in range(2):
                        off = side * 64
                        s_ps = sc_tile(f"s{side}_{b}_{h}_{kp}")
                        nc.tensor.matmul(s_ps[0:kw, 0:512],
                                         K12[b][off:off + 64, h, ksl],
                                         Q12[b][off:off + 64, h, 0:512],
                                         start=True, stop=True, skip_group_check=True)
                        nc.tensor.matmul(s_ps[0:kw, 512:577],
                                         K12[b][off:off + 64, h, ksl],
                                         Q12[b][off:off + 64, h, 512:577],
                                         start=True, stop=True, skip_group_check=True)
                        ss.append(s_ps)
                    for side in range(2):
                        nc.scalar.activation(out=e[0:kw, side, 0:577],
                                             in_=ss[side][0:kw, 0:577],
                                             func=AF.Exp, scale=0.125)
                    for side, c in ((0, c1), (1, c2)):
                        nc.tensor.matmul(c[:, 0:512], vaug[b][0:kw, kp, h, :],
                                         e[0:kw, side, 0:512],
                                         start=(kp == 0), stop=False,
                                         skip_group_check=True)
                        nc.tensor.matmul(c[:, 512:577], vaug[b][0:kw, kp, h, :],
                                         e[0:kw, side, 512:577],
                                         start=(kp == 0), stop=(kp == NT - 1),
                                         skip_group_check=True)
                nc.vector.tensor_copy(ctxf[b][0:65, h, 0:577], c1[0:65, 0:577])
                ch2 = cpool.tile([65, SQ], bf16, tag="csh2", name=f"csh2_{b}_{h}")
                csh2[b][h] = ch2
                nc.vector.tensor_copy(ch2[0:65, 0:577], c2[0:65, 0:577])
                g = h // 6
                if h % 3 == 2:
                    # batched side-1 denominator gather for heads h-2..h
                    r0 = 64 * g + (h % 6) - 2
                    nc.sync.dma_start(out=den_all[b][r0:r0 + 3, 0:577],
                                      in_=ctxf[b][64:65, h - 2:h + 1, 0:577])
                nc.gpsimd.dma_start(
                    out=den_all[b][64 * g + 32 + (h % 6):64 * g + 33 + (h % 6), 0:577],
                    in_=ch2[64:65, 0:577])
            return f

        for h in range(H):
            th.append(head_thunk(h))
        return th

    def tail_thunks(b, g):
        """Tail for head group g (heads 6g..6g+5): recip, combines, GN, apply,
        scratch write. Returns a thunk list."""
        th = []
        q0 = 64 * g

        def recip():
            r_all = spool.tile([102, SQ], f32, tag=f"rall{b}", bufs=2,
                               name=f"rall{b}_{g}")
            nc.vector.reciprocal(out=r_all[q0:q0 + 6, 0:577],
                                 in_=den_all[b][q0:q0 + 6, 0:577])
            nc.vector.reciprocal(out=r_all[q0 + 32:q0 + 38, 0:577],
                                 in_=den_all[b][q0 + 32:q0 + 38, 0:577])
            nc.vector.tensor_copy(r16[b][q0:q0 + 6, 0:577], r_all[q0:q0 + 6, 0:577])
            nc.vector.tensor_scalar(out=r16[b][q0 + 32:q0 + 38, 0:577],
                                    in0=r_all[q0 + 32:q0 + 38, 0:577],
                                    scalar1=-lam, scalar2=None, op0=OP.mult)
        th.append(recip)

        def combine(h):
            def f():
                rb = rpool.tile([64, 2, SQ], bf16, tag="rb", name=f"rb{b}_{h}")
                for side in range(2):
                    row = q0 + 32 * side + (h % 6)
                    nc.sync.dma_start(out=rb[:, side, 0:577],
                                      in_=bcast_ap(r16[b][row:row + 1, 0:577], 64))
                tmp = tpool.tile([64, SQ], bf16, tag="tmp", name=f"tmp{b}_{h}")
                ch = ctxf[b][0:64, h, 0:577]
                ch2 = csh2[b][h]
                nc.vector.tensor_tensor(out=tmp[:, 0:577], in0=ch, in1=rb[:, 0, 0:577],
                                        op=OP.mult)
                nc.vector.tensor_tensor(out=ch2[0:64, 0:577], in0=ch2[0:64, 0:577],
                                        in1=rb[:, 1, 0:577], op=OP.mult)
                nc.vector.scalar_tensor_tensor(out=ch, in0=tmp[:, 0:577], scalar=1.0,
                                               in1=ch2[0:64, 0:577],
                                               op0=OP.mult, op1=OP.add,
                                               accum_out=stats[b][:, h:h + 1])
                nc.vector.scalar_tensor_tensor(out=tmp[:, 0:577], in0=ch, scalar=1.0,
                                               in1=ch, op0=OP.mult, op1=OP.mult,
                                               accum_out=stats[b][:, H + h:H + h + 1])
            return f
        for h in range(6 * g, 6 * g + 6):
            th.append(combine(h))

        def gn_apply():
            sps = ctx_tile(f"gn{b}_{g}")
            stats_g = stats[b].rearrange("p (a c) -> p a c", a=2)[:, :, 6 * g:6 * g + 6]
            nc.tensor.matmul(sps[0:1, 0:12], ones64, stats_g, start=True, stop=True,
                             skip_group_check=True)
            ssb = spool.tile([1, 12], f32, tag=f"ssb{b}", bufs=2, name=f"ssb{b}_{g}")
            nc.vector.tensor_copy(ssb, sps[0:1, 0:12])
            mu = spool.tile([1, 6], f32, tag=f"mu{b}", bufs=2, name=f"mu{b}_{g}")
            nc.vector.tensor_scalar(out=mu, in0=ssb[0:1, 0:6], scalar1=1.0 / GN_N,
                                    scalar2=None, op0=OP.mult)
            musq = spool.tile([1, 6], f32, tag=f"musq{b}", bufs=2, name=f"musq{b}_{g}")
            nc.vector.tensor_tensor(out=musq, in0=mu, in1=mu, op=OP.mult)
            var = spool.tile([1, 6], f32, tag=f"var{b}", bufs=2, name=f"var{b}_{g}")
            nc.vector.scalar_tensor_tensor(out=var, in0=ssb[0:1, 6:12],
                                           scalar=1.0 / GN_N, in1=musq,
                                           op0=OP.mult, op1=OP.subtract)
            lnv = spool.tile([1, 6], f32, tag=f"lnv{b}", bufs=2, name=f"lnv{b}_{g}")
            nc.scalar.activation(out=lnv, in_=var, func=AF.Ln, bias=eps_t, scale=1.0)
            rstd = spool.tile([1, 6], f32, tag=f"rstd{b}", bufs=2, name=f"rstd{b}_{g}")
            nc.scalar.activation(out=rstd, in_=lnv, func=AF.Exp, scale=-0.5)
            mu_b = spool.tile([64, 6], f32, tag=f"mu_b{b}", bufs=2, name=f"mu_b{b}_{g}")
            rstd_b = spool.tile([64, 6], f32, tag=f"rstd_b{b}", bufs=2,
                                name=f"rstd_b{b}_{g}")
            nc.sync.dma_start(out=mu_b, in_=bcast_ap(mu[0:1, :], 64))
            nc.sync.dma_start(out=rstd_b, in_=bcast_ap(rstd[0:1, :], 64))
            scale_all = spool.tile([64, 6], f32, tag=f"scl{b}", bufs=2,
                                   name=f"scl{b}_{g}")
            nc.vector.tensor_tensor(out=scale_all, in0=rstd_b,
                                    in1=gn_wT[:, 6 * g:6 * g + 6], op=OP.mult)
            bias_all = spool.tile([64, 6], f32, tag=f"bia{b}", bufs=2,
                                  name=f"bia{b}_{g}")
            nc.vector.scalar_tensor_tensor(out=bias_all, in0=mu_b, scalar=-1.0,
                                           in1=scale_all, op0=OP.mult, op1=OP.mult)
            nc.vector.tensor_tensor(out=bias_all, in0=bias_all,
                                    in1=gn_bT[:, 6 * g:6 * g + 6], op=OP.add)
            for j in range(6):
                h = 6 * g + j
                nc.vector.tensor_scalar(out=ctxf[b][0:64, h, 0:577],
                                        in0=ctxf[b][0:64, h, 0:577],
                                        scalar1=scale_all[:, j:j + 1],
                                        scalar2=bias_all[:, j:j + 1],
                                        op0=OP.mult, op1=OP.add)
            nc.sync.dma_start(
                out=bass.AP(tensor=scr[b].tensor,
                            offset=scr[b].offset + g * 6 * 64 * S,
                            ap=[[S, 64], [64 * S, 6], [1, S]]),
                in_=ctxf[b][0:64, 6 * g:6 * g + 6, 0:577])
        th.append(gn_apply)
        return th

    def p3_thunks(b):
        """Output projection per seq tile. Tiles 0-1 only need scratch rows
        from head group 0; tiles 2-4 need group 1 too. Returns (early, late)."""
        cT = big.tile([128, NK, 640], bf16, tag="ctxTT", bufs=1, name=f"ctxTT{b}")

        def o_thunk(t):
            def f():
                sz = 128 if t < NT - 1 else LAST
                cn = xpool.tile([128, D], bf16, tag="xb", name=f"cn{b}_{t}")
                if sz < 128:
                    nc.vector.memset(cn, 0.0)
                nc.gpsimd.dma_start(out=cn[0:sz, :], in_=scr[b][t * 128:t * 128 + sz, :])
                tp = aux_tile(f"tpc{b}_{t}", (128, 1536), bf16)
                for k in range(NK):
                    nc.tensor.transpose(tp[:, k * 128:(k + 1) * 128],
                                        cn[:, k * 128:(k + 1) * 128], ident)
                cTt = cT[:, 0:NK, t * 128:(t + 1) * 128]
                nc.vector.tensor_copy(cTt, tp[:, 0:768].rearrange("p (k c) -> p k c", k=NK))
                o_ps = aux_tile(f"o{b}_{t}")
                for k in range(NK):
                    nc.tensor.matmul(o_ps[:, 0:512], cT[:, k, t * 128:(t + 1) * 128],
                                     WoB[:, k, 0:512], start=(k == 0), stop=False,
                                     skip_group_check=True)
                    nc.tensor.matmul(o_ps[:, 512:768], cT[:, k, t * 128:(t + 1) * 128],
                                     WoB[:, k, 512:768], start=(k == 0), stop=False,
                                     skip_group_check=True)
                nc.tensor.matmul(o_ps[:, 0:512], onesrow, bob[0:1, 0:512],
                                 start=False, stop=True, skip_group_check=True)
                nc.tensor.matmul(o_ps[:, 512:768], onesrow, bob[0:1, 512:768],
                                 start=False, stop=True, skip_group_check=True)
                ot = xpool.tile([128, D], f32, tag="ot", name=f"ot{b}_{t}")
                nc.vector.tensor_copy(ot[0:sz, :], o_ps[0:sz, 0:768])
                nc.sync.dma_start(out=out[b, t * 128:t * 128 + sz, :], in_=ot[0:sz, :])
            return f
        return [o_thunk(0), o_thunk(1)], [o_thunk(2), o_thunk(3), o_thunk(4)]

    def drive(primary, fillers, hook=None):
        n, m = len(primary), len(fillers)
        fi = 0
        for i, p in enumerate(primary):
            p()
            if hook is not None:
                hook(i)
            target = (i + 1) * m // n
            while fi < target:
                fillers[fi]()
                fi += 1
        while fi < m:
            fillers[fi]()
            fi += 1

    # ---------------- emission ----------------
    wq_prep, wv_prep, wk_prep, wo_prep = emit_w_prep()
    x0, q0, k0, v0 = p1_thunks(0)
    for t in x0:
        t()
    wq_prep()
    for t in q0:
        t()
    wv_prep()
    for t in v0:
        t()
    wk_prep()
    for t in k0:
        t()
    wo_prep()
    x1, q1, k1, v1 = p1_thunks(1)
    drive(attn_thunks(0), x1 + q1 + k1 + v1)

    # batch-0 tails + p3 fill attn(1); batch-1 group-0 tail fires mid-way
    tail0 = tail_thunks(0, 0) + tail_thunks(0, 1)
    p3e0, p3l0 = p3_thunks(0)
    tail1a = tail_thunks(1, 0)
    p3e1, p3l1 = p3_thunks(1)

    fired = [False]

    def hook(i):
        if i == 7 and not fired[0]:
            fired[0] = True
            for t in tail1a:
                t()

    drive(attn_thunks(1), tail0 + p3e0 + p3l0, hook=hook)
    if not fired[0]:
        for t in tail1a:
            t()
    for t in p3e1:
        t()
    for t in tail_thunks(1, 1):
        t()
    for t in p3l1:
        t()

    for p in (ps, drpool, spool, rpool, tpool, cpool, epool, xpool, wqk, big, sing):
        p.release()


_CACHE = {}
LAST_EXEC_NS = 0
LAST_TRACE = None


def _get_program(lam: float):
    key = round(float(lam), 8)
    if key not in _CACHE:
        _CACHE[key] = build_program(float(lam))
    return _CACHE[key]


def kernel(**inputs):
    x = np.ascontiguousarray(np.asarray(inputs["x"], dtype=np.float32))
    lam = float(np.asarray(inputs["lam"]))
    nc = _get_program(lam)
    names = ["Wq", "bq", "Wk", "bk", "Wv", "bv", "Wo", "bo", "gn_w", "gn_b"]
    shared = {n: np.ascontiguousarray(np.asarray(inputs[n], dtype=np.float32))
              for n in names}
    in_maps = []
    for c in range(N_CORES):
        m = dict(shared)
        m["x"] = x[c * BL:(c + 1) * BL]
        in_maps.append(m)
    res = bass_utils.run_bass_kernel_spmd(nc, in_maps, list(range(N_CORES)))
    global LAST_EXEC_NS, LAST_TRACE
    if getattr(res, "exec_time_ns", None):
        LAST_EXEC_NS = res.exec_time_ns
        LAST_TRACE = getattr(res, "instructions_and_trace", None)
    return np.concatenate([res.results[c]["out"] for c in range(N_CORES)], axis=0)
